# revision 31
# baseline (speedup 1.0000x reference)
"""Gated DeltaNet (Qwen3.5-style) forward on 8 Trainium2 NeuronCores.

Sharding: tensor-parallel over heads. Core i owns v-heads 4i..4i+3 and
k-heads 2i..2i+1 (GVA rep=2), both batch rows. Each core runs an identical
Bass program (SPMD) on its head-slice; no collectives.

Device algorithm (per core):
  depthwise causal conv(K=4) on PE (diagonal-weight matmuls) + SiLU on ACT
  -> chunked delta rule with chunk C=128: within-chunk unit-lower-triangular
  solve (I+L)^-1 via 3-term Neumann/Horner, inter-chunk recurrence on
  S[128,256] per (batch,kh) pair (two v-heads side by side). q/k l2-norms
  are folded into the decay exponents in log space.

v2 restructure vs baseline:
  - k|q interleaved conv output layout -> KK|KQ in one N=256 matmul/pair
  - decay row-term exp(-Gk(s)) applied via ACT-Exp bias column, so the
    per-chunk exponent tables need only a rank-1 [Gt|Gq] matmul (lhsT =
    static ones2); per-chunk table fetch is one contiguous DMA from a
    DRAM-staged gather built once at setup
  - q/k norms via N=1 ones-matmuls on squared tiles (no PE transposes)
  - pair-merged S[128,256]: pR/pQS are N=256 matmuls
  - Horner iter1 accumulates +R on PE (idbf matmul) and evacuates on ACT
    to balance DVE/ACT load
"""

import numpy as np

B, T = 2, 2048
HK, HV, DK, DV = 16, 32, 128, 128
CONV_DIM = 2 * HK * DK + HV * DV
KW = 4            # conv taps
C = 128           # chunk length
NCH = T // C      # 16 chunks
TPAD = T + 3      # left zero-pad per batch segment
NCORE = 8
LN_HALF_DK = 0.5 * float(np.log(DK))


# ---------------------------------------------------------------------------
# device program
# ---------------------------------------------------------------------------

def build_program(sim_compat=False):
    import concourse.bacc as bacc
    import concourse.tile as tile
    from concourse import mybir

    f32 = mybir.dt.float32
    bf16 = mybir.dt.bfloat16
    AF = mybir.ActivationFunctionType
    OP = mybir.AluOpType

    nc = bacc.Bacc("TRN2", target_bir_lowering=False, debug=False)

    dram = {}
    def din(name, shape, dt):
        dram[name] = nc.dram_tensor(name, shape, dt, kind="ExternalInput").ap()
        return dram[name]

    x_cm = din("x_cm", [8, 128, B * TPAD], bf16)
    wdiag = din("wdiag", [8, KW, 128, 128], bf16)
    # pf32 = [a_pk | b_pk | dtb | negea | cb | idf32 | wcol]
    pf32_in = din("pf32", [128, 426], f32)
    # pbf = [maskS | maskI | idbf | maskSI2]
    pbf_in = din("pbf", [128, 896], bf16)
    out_d = nc.dram_tensor("out", [B, T, 4 * DV], bf16, kind="ExternalOutput").ap()

    pairs = [(b_, kh) for b_ in range(B) for kh in range(2)]

    with tile.TileContext(nc) as tc:
        import contextlib
        ctx = contextlib.ExitStack()
        with ctx:
            consts = ctx.enter_context(tc.tile_pool(name="consts", bufs=1))
            ypool = ctx.enter_context(tc.tile_pool(name="ypool", bufs=1))
            spool = ctx.enter_context(tc.tile_pool(name="spool", bufs=2))
            scr = ctx.enter_context(tc.tile_pool(name="scr", bufs=4))
            work = ctx.enter_context(tc.tile_pool(name="work", bufs=3))
            dscr = ctx.enter_context(
                tc.tile_pool(name="dscr", bufs=1, space="DRAM"))

            # ---------------- constants in ----------------
            pf32 = consts.tile([128, 426], f32, tag="pf32")
            nc.sync.dma_start(pf32, pf32_in)
            pbf = consts.tile([128, 896], bf16, tag="pbf")
            nc.sync.dma_start(pbf, pbf_in)
            wd = consts.tile([128, 8, KW, 128], bf16, tag="wd")
            nc.sync.dma_start(wd, wdiag.rearrange("t j p c -> p t j c"))
            a_pk = pf32[:, 0:128]
            b_pk = pf32[:, 128:256]
            dtb = pf32[:, 256:257]
            negea = pf32[:, 257:258]
            cbt = pf32[:, 258:266]
            idf = pf32[:, 266:394]
            wcol = pf32[:, 394:426]  # per-partition conv taps, col = ct*4+j
            idbf = pbf[:, 256:384]
            maskSI2 = pbf[:, 384:896]
            ones_col = consts.tile([128, 1], bf16, tag="ones_col")
            nc.vector.memset(ones_col, 1.0)
            ones2 = consts.tile([2, 128], bf16, tag="ones2")
            nc.vector.memset(ones2, 1.0)
            ones1 = consts.tile([1, 128], bf16, tag="ones1")
            nc.vector.memset(ones1, 1.0)
            zeros_s = consts.tile([128, 128], f32, tag="zeros_s")
            nc.vector.memset(zeros_s, 0.0)
            epsc = consts.tile([128, 1], f32, tag="epsc")
            nc.vector.memset(epsc, 1e-6)

            # ---------------- G math part A (only needs pf32) -------------
            ea_t = consts.tile([128, 128], f32, tag="ea_t")
            nc.scalar.activation(ea_t, a_pk, AF.Exp, bias=dtb, scale=1.0)
            spa = consts.tile([128, 128], f32, tag="spa")
            nc.scalar.activation(spa, ea_t, AF.Ln, bias=1.0, scale=1.0)
            g_pk = consts.tile([128, 128], f32, tag="g_pk")
            nc.vector.tensor_scalar_mul(g_pk, spa, negea)
            G = consts.tile([128, 128], f32, tag="G")
            nc.vector.tensor_tensor_scan(
                G, g_pk, zeros_s, 0.0, op0=OP.add, op1=OP.add)
            eb_t = consts.tile([128, 128], f32, tag="eb_t")
            nc.scalar.activation(eb_t, b_pk, AF.Exp, scale=-1.0)
            spnb = consts.tile([128, 128], f32, tag="spnb")  # softplus(-b) = -ln(beta)
            nc.scalar.activation(spnb, eb_t, AF.Ln, bias=1.0, scale=1.0)
            # beta = sigmoid(b) = 1/(1+exp(-b)) on DVE (keeps ACT on one LUT)
            ebp1 = scr.tile([128, 128], f32, tag="ebp1")
            nc.vector.tensor_scalar_add(ebp1, eb_t, 1.0)
            beta_pk = consts.tile([128, 128], f32, tag="beta_pk")
            nc.vector.reciprocal(beta_pk, ebp1)

            # ---------------- conv + silu + norms ----------------
            # ykq[kh]: [128, 8192] interleaved per (b, chunk): [k(128) | q(128)]
            # ytv[hl]: [128, 4096] plain
            ykq = [ypool.tile([128, B * NCH * 256], bf16, tag=f"ykq{kh}",
                              name=f"ykq{kh}") for kh in range(2)]
            ytv = [ypool.tile([128, B * T], bf16, tag=f"ytv{hl}",
                              name=f"ytv{hl}") for hl in range(4)]

            pset_stack = contextlib.ExitStack()
            pset = pset_stack.enter_context(
                tc.tile_pool(name="pset", bufs=1, space="PSUM"))
            # norm accumulator: cols 0-63 = k (j), 64-127 = q (64+j)
            nacc = pset.tile([128, 128], f32, tag="nacc")

            def conv_ct_dve(xin, ct):
                # depthwise conv on DVE via per-partition tensor_scalar
                # chains (v heads only; PE stays on k/q + chunk work)
                xs = xin.tile([128, B * TPAD], bf16, tag="xs", name="xs")
                nc.sync.dma_start(xs, x_cm[ct])
                for b_ in range(B):
                    for blk in range(4):
                        base = b_ * TPAD + blk * 512
                        acc = xin.tile([128, 512], bf16, tag="vacc",
                                       bufs=8, name="acc")
                        nc.vector.tensor_scalar_mul(
                            acc, xs[:, base: base + 512],
                            wcol[:, ct * 4: ct * 4 + 1])
                        for j in range(1, KW):
                            acc2 = xin.tile([128, 512], bf16, tag="vacc",
                                            bufs=8, name="acc2")
                            nc.vector.scalar_tensor_tensor(
                                acc2, xs[:, base + j: base + j + 512],
                                wcol[:, ct * 4 + j: ct * 4 + j + 1], acc,
                                op0=OP.mult, op1=OP.add)
                            acc = acc2
                        ydst = ytv[ct - 4][:, b_ * T + blk * 512:
                                           b_ * T + (blk + 1) * 512]
                        if sim_compat:
                            zc = xin.tile([128, 512], bf16, tag="zc",
                                          name="zc")
                            nc.scalar.activation(
                                zc, acc, AF.Identity,
                                bias=cbt[:, ct:ct + 1], scale=1.0)
                            sg = xin.tile([128, 512], bf16, tag="sg",
                                          name="sg")
                            nc.scalar.activation(sg, zc, AF.Sigmoid)
                            nc.vector.tensor_mul(ydst, zc, sg)
                        else:
                            nc.scalar.activation(
                                ydst, acc, AF.Silu,
                                bias=cbt[:, ct:ct + 1], scale=1.0)

            def conv_ct(xin, pcvp, ct):
                xs = xin.tile([128, B * TPAD], bf16, tag="xs", name="xs")
                nc.sync.dma_start(xs, x_cm[ct])
                for b_ in range(B):
                    for bp in range(2):
                        pcv = pcvp.tile([128, 1024], f32, tag="pconv",
                                        name="pcv")
                        for h in range(2):
                            base = b_ * TPAD + bp * 1024 + h * 512
                            for j in range(KW):
                                nc.tensor.matmul(
                                    pcv[:, h * 512:(h + 1) * 512],
                                    wd[:, ct, j, :],
                                    xs[:, base + j: base + j + 512],
                                    start=(j == 0), stop=(j == KW - 1))
                        if ct >= 4:
                            dst = ytv[ct - 4][:, b_ * T + bp * 1024:
                                              b_ * T + (bp + 1) * 1024]
                            src = pcv
                        else:
                            kh = ct - 2 if ct >= 2 else ct
                            half = 0 if ct >= 2 else 1  # k -> 0, q -> 1
                            seg = ykq[kh][:, (b_ * 16 + 8 * bp) * 256:
                                          (b_ * 16 + 8 * bp + 8) * 256]
                            dst = seg.rearrange(
                                "p (c two t) -> p c two t", c=8, two=2,
                                t=128)[:, :, half:half + 1, :]
                            src = pcv.rearrange(
                                "p (c o t) -> p c o t", c=8, o=1, t=128)
                        if sim_compat:
                            zc = xin.tile([128, 1024], bf16, tag="zc",
                                          name="zc")
                            nc.scalar.activation(
                                zc, pcv, AF.Identity,
                                bias=cbt[:, ct:ct + 1], scale=1.0)
                            sg = xin.tile([128, 1024], bf16, tag="sg",
                                          name="sg")
                            nc.scalar.activation(sg, zc, AF.Sigmoid)
                            nc.vector.tensor_mul(
                                dst, zc.rearrange("p (c o t) -> p c o t",
                                                  c=8, o=1, t=128)
                                if ct < 4 else zc,
                                sg.rearrange("p (c o t) -> p c o t",
                                             c=8, o=1, t=128)
                                if ct < 4 else sg)
                        else:
                            nc.scalar.activation(
                                dst, src, AF.Silu,
                                bias=cbt[:, ct:ct + 1], scale=1.0)

            def norms_kh(ysqp, kh):
                # squares + N=1 ones-matmul column sums into kacc/qacc psum
                for b_ in range(B):
                    ysq = ysqp.tile([128, 4096], bf16, tag="ysq", name="ysq")
                    nc.scalar.activation(
                        ysq, ykq[kh][:, b_ * 4096:(b_ + 1) * 4096], AF.Square)
                    for c_ in range(NCH):
                        j = b_ * 32 + kh * 16 + c_
                        nc.tensor.matmul(
                            nacc[:, j:j + 1],
                            ysq[:, c_ * 256: c_ * 256 + 128], ones_col,
                            start=True, stop=True)
                        nc.tensor.matmul(
                            nacc[:, 64 + j:64 + j + 1],
                            ysq[:, c_ * 256 + 128: c_ * 256 + 256], ones_col,
                            start=True, stop=True)

            with tc.tile_pool(name="xin", bufs=2) as xin, \
                    tc.tile_pool(name="ysqp", bufs=2) as ysqp, \
                    tc.tile_pool(name="pcv", bufs=2, space="PSUM") as pcvp:
                for ct in (2, 0):
                    conv_ct(xin, pcvp, ct)
                norms_kh(ysqp, 0)
                for ct in (3, 1):
                    conv_ct(xin, pcvp, ct)
                norms_kh(ysqp, 1)
                for ct in (4, 5, 6, 7):
                    conv_ct_dve(xin, ct)

            # ---------------- ln(norms) + packed expand ----------------
            lnkq = scr.tile([128, 128], f32, tag="lnkq")
            nc.scalar.activation(lnkq, nacc, AF.Ln, bias=epsc, scale=1.0)
            # nacc layout [t, col=b*32+kh*16+c (+64 for q)] -> transpose,
            # then expand dups to packed [row=inst*16+c, t]
            Lk_pk = consts.tile([128, 128], f32, tag="Lk_pk")
            Lq_pk = consts.tile([128, 128], f32, tag="Lq_pk")
            pt = pset.tile([128, 128], f32, tag="ptr", bufs=2)
            nc.tensor.transpose(pt, lnkq, idf)
            st = scr.tile([128, 128], f32, tag="snorm")
            nc.vector.tensor_copy(st, pt)
            for qoff, dst in ((0, Lk_pk), (64, Lq_pk)):
                for b_ in range(B):
                    for kh in range(2):
                        for dup in range(2):
                            inst = b_ * 4 + kh * 2 + dup
                            nc.sync.dma_start(
                                dst[inst * 16: inst * 16 + 16, :],
                                st[qoff + b_ * 32 + kh * 16:
                                   qoff + b_ * 32 + kh * 16 + 16, :])

            # ---------------- G math part B (needs norms) ----------------
            t1 = scr.tile([128, 128], f32, tag="t1")     # G - 0.5*Lk
            nc.vector.scalar_tensor_tensor(
                t1, Lk_pk, -0.5, G, op0=OP.mult, op1=OP.add)
            Gt = consts.tile([128, 128], f32, tag="Gt")  # G + ln(beta) - 0.5*Lk
            nc.vector.tensor_sub(Gt, t1, spnb)
            Gk = consts.tile([128, 128], f32, tag="Gk")  # G + 0.5*Lk
            nc.vector.scalar_tensor_tensor(
                Gk, Lk_pk, 0.5, G, op0=OP.mult, op1=OP.add)
            t2 = scr.tile([128, 128], f32, tag="t2")
            nc.vector.scalar_tensor_tensor(
                t2, Lq_pk, -0.5, G, op0=OP.mult, op1=OP.add)
            Gq = consts.tile([128, 128], f32, tag="Gq")
            nc.vector.tensor_scalar_add(Gq, t2, -LN_HALF_DK)
            expGt = scr.tile([128, 128], f32, tag="expGt")
            nc.scalar.activation(expGt, Gt, AF.Exp)
            negexpGt = consts.tile([128, 128], f32, tag="negexpGt")
            nc.vector.tensor_scalar_mul(negexpGt, expGt, -1.0)
            decrn = consts.tile([128, 128], f32, tag="decrn")  # exp(G_C - Gk)
            nc.scalar.activation(decrn, Gk, AF.Exp, bias=G[:, 127:128],
                                 scale=-1.0)
            eGqn = consts.tile([128, 128], f32, tag="eGqn")    # exp(Gq)
            nc.scalar.activation(eGqn, Gq, AF.Exp)

            # eGC broadcast [128, 128] (col r = exp(G_C(r)) replicated)
            eGCc = scr.tile([128, 1], bf16, tag="eGCc")
            nc.scalar.activation(eGCc, G[:, 127:128], AF.Exp)
            pt1 = pset.tile([1, 128], bf16, tag="ptr", bufs=2)
            nc.tensor.transpose(pt1, eGCc, idbf)
            eGCrow = scr.tile([1, 128], bf16, tag="eGCrow")
            nc.vector.tensor_copy(eGCrow, pt1)
            pb = pset.tile([128, 128], f32, tag="ptr", bufs=2)
            nc.tensor.matmul(pb, ones1, eGCrow, start=True, stop=True)
            eGCb = consts.tile([128, 128], f32, tag="eGCb")
            nc.vector.tensor_copy(eGCb, pb)

            # transposed per-time tables: [128(t), 128(col=r)]
            beta_T = consts.tile([128, 128], f32, tag="beta_T")
            negeGt_T = consts.tile([128, 128], f32, tag="negeGt_T")
            decrn_T = consts.tile([128, 128], f32, tag="decrn_T")
            eGqn_T = consts.tile([128, 128], f32, tag="eGqn_T")
            negGk_T = consts.tile([128, 128], f32, tag="negGk_T")
            for src_t, dst, sc in ((beta_pk, beta_T, 1.0),
                                   (negexpGt, negeGt_T, 1.0),
                                   (decrn, decrn_T, 1.0),
                                   (eGqn, eGqn_T, 1.0),
                                   (Gk, negGk_T, -1.0)):
                pt2 = pset.tile([128, 128], f32, tag="ptr", bufs=2)
                nc.tensor.transpose(pt2, src_t, idf)
                if sc == 1.0:
                    nc.vector.tensor_copy(dst, pt2)
                else:
                    nc.vector.tensor_scalar_mul(dst, pt2, sc)

            # hi/lo bf16 splits of Gt/Gq
            def hilo(src_t, nm):
                hi = consts.tile([128, 128], bf16, name=f"{nm}h", tag=f"{nm}h")
                nc.vector.tensor_copy(hi, src_t)
                lo = consts.tile([128, 128], bf16, name=f"{nm}l", tag=f"{nm}l")
                nc.vector.tensor_sub(lo, src_t, hi)
                return hi, lo

            Gth, Gtl = hilo(Gt, "Gt")
            Gqh, Gql = hilo(Gq, "Gq")

            # flatten hi/lo tables to [2(hi,lo), (c,i,w,t)] in SBUF so the
            # per-chunk per-pair rank-1 matmul rhs is one CONTIGUOUS
            # [2, 512] slice ([Gt_i0|Gq_i0|Gt_i1|Gq_i1]).  Own pool
            # (opened after conv scratch closed) so the stack allocator
            # reuses the freed conv region.
            flatp = ctx.enter_context(tc.tile_pool(name="flatp", bufs=1))
            GtGq_flat = flatp.tile([2, NCH * 8 * 2 * 128], bf16,
                                   tag="GtGq_flat", name="GtGq_flat")
            fl_v = GtGq_flat.rearrange("p (c i w t) -> p c i w t",
                                       c=NCH, i=8, w=2)
            for row, w, src_t in ((0, 0, Gth), (1, 0, Gtl),
                                  (0, 1, Gqh), (1, 1, Gql)):
                for i in range(8):
                    nc.sync.dma_start(
                        fl_v[row:row + 1, :, i:i + 1, w:w + 1, :],
                        src_t[i * 16:(i + 1) * 16, :])

            # ---------------- state init ----------------
            S = {}
            for p in range(4):
                s0 = consts.tile([128, 256], bf16, name=f"S{p}", tag=f"S{p}")
                nc.vector.memset(s0, 0.0)
                S[p] = s0

            pset_stack.close()

            # ---------------- chunk loop ----------------
            ps = ctx.enter_context(tc.tile_pool(name="ps", bufs=1,
                                                space="PSUM"))

            def rof(inst, c_):
                return inst * 16 + c_

            for c_ in range(NCH):
                def kqslice(kh, b_, width, off=0):
                    base = (b_ * 16 + c_) * 256 + off
                    return ykq[kh][:, base: base + width]

                # P1: decay psums [L0|A0|L1|A1] + E2
                pDs = []
                for p, (b_, kh) in enumerate(pairs):
                    i0 = b_ * 4 + kh * 2
                    pD = ps.tile([128, 512], f32, tag="pD", bufs=2,
                                 name="pD")
                    nc.tensor.matmul(pD, idbf, maskSI2,
                                     start=True, stop=False)
                    nc.tensor.matmul(
                        pD, ones2,
                        GtGq_flat[:, c_ * 2048 + i0 * 256:
                                  c_ * 2048 + (i0 + 2) * 256],
                        start=False, stop=True)
                    pDs.append(pD)
                E2 = {}
                for p, (b_, kh) in enumerate(pairs):
                    for dup in range(2):
                        inst = b_ * 4 + kh * 2 + dup
                        r = rof(inst, c_)
                        e2 = work.tile([128, 256], bf16, tag="e2", bufs=8,
                                       name="e2")
                        nc.scalar.activation(
                            e2, pDs[p][:, dup * 256:(dup + 1) * 256],
                            AF.Exp, bias=negGk_T[:, r:r + 1], scale=1.0)
                        E2[inst] = e2

                # P2: KKQ + pR psums (packed per-pair bank) + KKQ evac
                # pX layout per use:
                #   pkr:  pk [0:256], pR [256:512]
                #   pU:   pU1_d0 [0:128], pU1_d1 [128:256],
                #         pU2_d0 [256:384], pU2_d1 [384:512]
                #   pout: pQS [0:256], pAU_d0 [256:384], pAU_d1 [384:512]
                #   pS:   4 insts x [128]
                kq_sb = {}
                pkr = []
                for p, (b_, kh) in enumerate(pairs):
                    px = ps.tile([128, 512], f32, tag="pX", bufs=4,
                                 name="pkr")
                    pkr.append(px)
                    nc.tensor.matmul(px[:, 0:256], kqslice(kh, b_, 128),
                                     kqslice(kh, b_, 256),
                                     start=True, stop=True)
                    # negated evac so nl is a plain tensor_tensor multiply
                    kq = work.tile([128, 256], bf16, tag="kq", bufs=4,
                                   name="kq")
                    nc.scalar.mul(kq, px[:, 0:256], -1.0)
                    kq_sb[p] = kq

                # P3: nl = E2 * (-[KK|KQ])  (bf16 2x DVE)
                nL = {}
                for p, (b_, kh) in enumerate(pairs):
                    for dup in range(2):
                        inst = b_ * 4 + kh * 2 + dup
                        nl = work.tile([128, 256], bf16, tag="nl", bufs=8,
                                       name="nl")
                        nc.vector.tensor_mul(nl, E2[inst], kq_sb[p])
                        nL[inst] = nl

                # P4: k + v transposes (packed bf16 bank) + ktl/bv (ACT)
                # pTa: pkt p0..p3 at [p*128], pVT inst0..3 at [512+hl*128]
                # pTb: pVT inst4..7 at [hl*128]
                pTa = ps.tile([128, 1024], bf16, tag="pT", bufs=2,
                              name="pTa")
                pTb = ps.tile([128, 1024], bf16, tag="pT", bufs=2,
                              name="pTb")
                ktl = {}
                for p, (b_, kh) in enumerate(pairs):
                    pkt = pTa[:, p * 128:(p + 1) * 128]
                    nc.tensor.transpose(pkt, kqslice(kh, b_, 128), idbf)
                    for dup in range(2):
                        inst = b_ * 4 + kh * 2 + dup
                        r = rof(inst, c_)
                        kt = work.tile([128, 128], bf16, tag="ktl", bufs=8,
                                       name="kt")
                        nc.scalar.mul(kt, pkt, decrn_T[:, r:r + 1])
                        ktl[inst] = kt

                bv = {}
                for b_ in range(B):
                    for hl in range(4):
                        inst = b_ * 4 + hl
                        r = rof(inst, c_)
                        if b_ == 0:
                            pVT = pTa[:, 512 + hl * 128: 640 + hl * 128]
                        else:
                            pVT = pTb[:, hl * 128:(hl + 1) * 128]
                        nc.tensor.transpose(
                            pVT, ytv[hl][:, b_ * T + c_ * C:
                                         b_ * T + (c_ + 1) * C], idbf)
                        b1 = work.tile([128, 128], bf16, tag="bv", bufs=8,
                                       name="b1")
                        nc.scalar.mul(b1, pVT, beta_T[:, r:r + 1])
                        bv[inst] = b1

                # P6: pR (pair, into pkr bank) + R (DVE, pair tile halves)
                for p, (b_, kh) in enumerate(pairs):
                    nc.tensor.matmul(pkr[p][:, 256:512],
                                     kqslice(kh, b_, 128), S[p],
                                     start=True, stop=True)
                R = {}
                Rp = {}
                for p, (b_, kh) in enumerate(pairs):
                    Rpair = work.tile([128, 256], bf16, tag="R", bufs=8,
                                      name="Rpair")
                    Rp[p] = Rpair
                    for dup in range(2):
                        inst = b_ * 4 + kh * 2 + dup
                        r = rof(inst, c_)
                        nc.vector.scalar_tensor_tensor(
                            Rpair[:, dup * 128:(dup + 1) * 128],
                            pkr[p][:, 256 + dup * 128:
                                   256 + (dup + 1) * 128],
                            negeGt_T[:, r:r + 1], bv[inst],
                            op0=OP.mult, op1=OP.add)
                        R[inst] = Rpair[:, dup * 128:(dup + 1) * 128]

                # P7/P8: Horner — U1 = nL@R + R, U2 = nL@U1 + R
                # (pair-wide DVE stt evacuations)
                pUs = []
                U1 = {}
                for p, (b_, kh) in enumerate(pairs):
                    pU = ps.tile([128, 512], f32, tag="pX", bufs=4,
                                 name="pU")
                    pUs.append(pU)
                    for dup in range(2):
                        inst = b_ * 4 + kh * 2 + dup
                        nc.tensor.matmul(pU[:, dup * 128:(dup + 1) * 128],
                                         nL[inst][:, 0:128], R[inst],
                                         start=True, stop=True)
                    u1p = work.tile([128, 256], bf16, tag="U1", bufs=8,
                                    name="u1p")
                    nc.vector.scalar_tensor_tensor(
                        u1p, pU[:, 0:256], 1.0, Rp[p],
                        op0=OP.mult, op1=OP.add)
                    for dup in range(2):
                        inst = b_ * 4 + kh * 2 + dup
                        U1[inst] = u1p[:, dup * 128:(dup + 1) * 128]
                U2 = {}
                for p, (b_, kh) in enumerate(pairs):
                    for dup in range(2):
                        inst = b_ * 4 + kh * 2 + dup
                        nc.tensor.matmul(
                            pUs[p][:, 256 + dup * 128: 256 + (dup + 1) * 128],
                            nL[inst][:, 0:128], U1[inst],
                            start=True, stop=True)
                    u2p = work.tile([128, 256], bf16, tag="U2", bufs=8,
                                    name="u2p")
                    nc.vector.scalar_tensor_tensor(
                        u2p, pUs[p][:, 256:512], 1.0, Rp[p],
                        op0=OP.mult, op1=OP.add)
                    for dup in range(2):
                        inst = b_ * 4 + kh * 2 + dup
                        U2[inst] = u2p[:, dup * 128:(dup + 1) * 128]

                # P9: output O = eGqn*(q S) - (-A)U  (pout bank per pair)
                pouts = []
                for p, (b_, kh) in enumerate(pairs):
                    po = ps.tile([128, 512], f32, tag="pX", bufs=4,
                                 name="pout")
                    pouts.append(po)
                    nc.tensor.matmul(po[:, 0:256],
                                     kqslice(kh, b_, 128, off=128),
                                     S[p], start=True, stop=True)
                    for dup in range(2):
                        inst = b_ * 4 + kh * 2 + dup
                        nc.tensor.matmul(
                            po[:, 256 + dup * 128: 256 + (dup + 1) * 128],
                            nL[inst][:, 128:256], U2[inst],
                            start=True, stop=True)
                for p, (b_, kh) in enumerate(pairs):
                    aup = work.tile([128, 256], bf16, tag="AU", bufs=4,
                                    name="aup")
                    nc.scalar.copy(aup, pouts[p][:, 256:512])
                    for dup in range(2):
                        hl = kh * 2 + dup
                        inst = b_ * 4 + hl
                        r = rof(inst, c_)
                        O_sb = work.tile([128, 128], bf16, tag="O", bufs=4,
                                         name="O_sb")
                        nc.vector.scalar_tensor_tensor(
                            O_sb, pouts[p][:, dup * 128:(dup + 1) * 128],
                            eGqn_T[:, r:r + 1],
                            aup[:, dup * 128:(dup + 1) * 128],
                            op0=OP.mult, op1=OP.subtract)
                        nc.sync.dma_start(
                            out_d[b_, c_ * C:(c_ + 1) * C,
                                  hl * DV:(hl + 1) * DV], O_sb)

                # P10: state update (pS packed 4-per-bank, 2 banks/chunk)
                Snew = {}
                for p, (b_, kh) in enumerate(pairs):
                    Snew[p] = spool.tile([128, 256], bf16, tag=f"Sn{p}",
                                         name="Snew")
                for half in range(2):
                    pSb = ps.tile([128, 512], f32, tag="pX", bufs=4,
                                  name="pSb")
                    for pi in range(2):
                        p = half * 2 + pi
                        b_, kh = pairs[p]
                        for dup in range(2):
                            inst = b_ * 4 + kh * 2 + dup
                            r = rof(inst, c_)
                            sl = pSb[:, (pi * 2 + dup) * 128:
                                     (pi * 2 + dup + 1) * 128]
                            nc.tensor.matmul(sl, ktl[inst], U2[inst],
                                             start=True, stop=True)
                            nc.vector.scalar_tensor_tensor(
                                Snew[p][:, dup * 128:(dup + 1) * 128],
                                S[p][:, dup * 128:(dup + 1) * 128],
                                eGCb[:, r:r + 1], sl,
                                op0=OP.mult, op1=OP.add)
                for p in range(4):
                    S[p] = Snew[p]
    nc.compile()
    return nc


# ---------------------------------------------------------------------------
# host-side prep
# ---------------------------------------------------------------------------

def host_inputs(core, mixed_qkv, a, b, conv_weight, conv_bias, A_log, dt_bias):
    import ml_dtypes
    bf = ml_dtypes.bfloat16
    f32 = np.float32

    chsel = np.r_[256 * core: 256 * core + 256,
                  2048 + 256 * core: 2048 + 256 * core + 256,
                  4096 + 512 * core: 4096 + 512 * core + 512]
    xs = mixed_qkv[:, :, chsel]                     # [B,T,1024]
    ws = conv_weight[chsel].astype(f32)             # [1024,4]
    cbs = conv_bias[chsel].astype(f32)

    x_cm = np.zeros((8, 128, B * TPAD), dtype=bf)
    xt = np.ascontiguousarray(xs.transpose(2, 0, 1))  # [1024,B,T]
    for b_ in range(B):
        x_cm[:, :, b_ * TPAD + 3: (b_ + 1) * TPAD] = \
            xt[:, b_, :].reshape(8, 128, T).astype(bf)

    wdiag = np.zeros((8, KW, 128, 128), dtype=bf)
    wr = ws.reshape(8, 128, KW)
    di = np.arange(128)
    for t_ in range(8):
        for j in range(KW):
            wdiag[t_, j, di, di] = wr[t_, :, j].astype(bf)

    cb_t = np.ascontiguousarray(cbs.reshape(8, 128).T).astype(f32)  # [128,8]

    a_pk = np.zeros((128, 128), f32)
    b_pk = np.zeros((128, 128), f32)
    dtb = np.zeros((128, 1), f32)
    negea = np.zeros((128, 1), f32)
    for b_ in range(B):
        for hl in range(4):
            hg = 4 * core + hl
            inst = b_ * 4 + hl
            a_pk[inst * 16: (inst + 1) * 16] = a[b_, :, hg].reshape(NCH, C)
            b_pk[inst * 16: (inst + 1) * 16] = b[b_, :, hg].reshape(NCH, C)
            dtb[inst * 16: (inst + 1) * 16] = dt_bias[hg]
            negea[inst * 16: (inst + 1) * 16] = -np.exp(A_log[hg])

    idx = np.arange(C)
    maskS = np.where(idx[:, None] < idx[None, :], 0.0, -1e6).astype(bf)
    maskI = np.where(idx[:, None] <= idx[None, :], 0.0, -1e6).astype(bf)
    idbf = np.eye(128, dtype=bf)
    idf32 = np.eye(128, dtype=f32)

    wcol_t = np.ascontiguousarray(
        wr.transpose(1, 0, 2).reshape(128, 32)).astype(f32)  # [p, ct*4+j]
    pf32 = np.concatenate(
        [a_pk, b_pk, dtb, negea, cb_t, idf32, wcol_t], axis=1).astype(f32)
    # maskSI2 ordered [S|I|S|I] to match the pD layout [L0|A0|L1|A1]
    pbf = np.ascontiguousarray(np.concatenate(
        [maskS, maskI, idbf, maskS, maskI, maskS, maskI], axis=1)).astype(bf)
    return {"x_cm": x_cm, "wdiag": wdiag, "pf32": pf32, "pbf": pbf}


_CACHED = {}


def _get_program():
    if "nc" not in _CACHED:
        _CACHED["nc"] = build_program()
    return _CACHED["nc"]


def kernel(mixed_qkv, a, b, conv_weight, conv_bias, A_log, dt_bias, trace=False):
    f32 = np.float32
    mixed_qkv = np.asarray(mixed_qkv, f32)
    a = np.asarray(a, f32)
    b = np.asarray(b, f32)
    conv_weight = np.asarray(conv_weight, f32)
    conv_bias = np.asarray(conv_bias, f32)
    A_log = np.asarray(A_log, f32)
    dt_bias = np.asarray(dt_bias, f32)

    from concourse import bass_utils

    nc = _get_program()
    in_maps = [host_inputs(core, mixed_qkv, a, b, conv_weight, conv_bias,
                           A_log, dt_bias) for core in range(NCORE)]
    res = bass_utils.run_bass_kernel_spmd(
        nc, in_maps, core_ids=list(range(NCORE)), trace=trace)

    out = np.empty((B, T, HV * DV), f32)
    for core in range(NCORE):
        out[:, :, 512 * core: 512 * (core + 1)] = \
            res.results[core]["out"].astype(f32)
    if trace:
        return out, res
    return out


# revision 32
# speedup vs baseline: 1.1149x; 1.1149x over previous
"""Gated DeltaNet (Qwen3.5-style) forward on 8 Trainium2 NeuronCores.

Sharding: tensor-parallel over heads. Core i owns v-heads 4i..4i+3 and
k-heads 2i..2i+1 (GVA rep=2), both batch rows. Each core runs an identical
Bass program (SPMD) on its head-slice; no collectives.

Device algorithm (per core):
  depthwise causal conv(K=4) on PE (diagonal-weight matmuls) + SiLU on ACT
  -> chunked delta rule with chunk C=128: within-chunk unit-lower-triangular
  solve (I+L)^-1 via 3-term Neumann/Horner, inter-chunk recurrence on
  S[128,256] per (batch,kh) pair (two v-heads side by side). q/k l2-norms
  are folded into the decay exponents in log space.

v2 restructure vs baseline:
  - k|q interleaved conv output layout -> KK|KQ in one N=256 matmul/pair
  - decay row-term exp(-Gk(s)) applied via ACT-Exp bias column, so the
    per-chunk exponent tables need only a rank-1 [Gt|Gq] matmul (lhsT =
    static ones2); per-chunk table fetch is one contiguous DMA from a
    DRAM-staged gather built once at setup
  - q/k norms via N=1 ones-matmuls on squared tiles (no PE transposes)
  - pair-merged S[128,256]: pR/pQS are N=256 matmuls
  - Horner iter1 accumulates +R on PE (idbf matmul) and evacuates on ACT
    to balance DVE/ACT load
"""

import numpy as np

B, T = 2, 2048
HK, HV, DK, DV = 16, 32, 128, 128
CONV_DIM = 2 * HK * DK + HV * DV
KW = 4            # conv taps
C = 128           # chunk length
NCH = T // C      # 16 chunks
TPAD = T + 3      # left zero-pad per batch segment
NCORE = 8
LN_HALF_DK = 0.5 * float(np.log(DK))


# ---------------------------------------------------------------------------
# device program
# ---------------------------------------------------------------------------

def build_program(sim_compat=False):
    import concourse.bacc as bacc
    import concourse.tile as tile
    from concourse import mybir

    f32 = mybir.dt.float32
    bf16 = mybir.dt.bfloat16
    AF = mybir.ActivationFunctionType
    OP = mybir.AluOpType

    nc = bacc.Bacc("TRN2", target_bir_lowering=False, debug=False)

    dram = {}
    def din(name, shape, dt):
        dram[name] = nc.dram_tensor(name, shape, dt, kind="ExternalInput").ap()
        return dram[name]

    x_cm = din("x_cm", [8, 128, B * TPAD], bf16)
    wdiag = din("wdiag", [8, KW, 128, 128], bf16)
    # pf32 = [a_pk | b_pk | dtb | negea | cb | idf32 | wcol]
    pf32_in = din("pf32", [128, 426], f32)
    # pbf = [maskS | maskI | idbf | maskSI2]
    pbf_in = din("pbf", [128, 896], bf16)
    out_d = nc.dram_tensor("out", [B, T, 4 * DV], bf16, kind="ExternalOutput").ap()

    pairs = [(b_, kh) for b_ in range(B) for kh in range(2)]

    with tile.TileContext(nc) as tc:
        import contextlib
        ctx = contextlib.ExitStack()
        with ctx:
            consts = ctx.enter_context(tc.tile_pool(name="consts", bufs=1))
            ypool = ctx.enter_context(tc.tile_pool(name="ypool", bufs=1))
            spool = ctx.enter_context(tc.tile_pool(name="spool", bufs=2))
            scr = ctx.enter_context(tc.tile_pool(name="scr", bufs=4))
            work = ctx.enter_context(tc.tile_pool(name="work", bufs=3))
            dscr = ctx.enter_context(
                tc.tile_pool(name="dscr", bufs=1, space="DRAM"))

            # ---------------- constants in ----------------
            pf32 = consts.tile([128, 426], f32, tag="pf32")
            nc.sync.dma_start(pf32, pf32_in)
            pbf = consts.tile([128, 896], bf16, tag="pbf")
            nc.sync.dma_start(pbf, pbf_in)
            wd = consts.tile([128, 8, KW, 128], bf16, tag="wd")
            nc.sync.dma_start(wd, wdiag.rearrange("t j p c -> p t j c"))
            a_pk = pf32[:, 0:128]
            b_pk = pf32[:, 128:256]
            dtb = pf32[:, 256:257]
            negea = pf32[:, 257:258]
            cbt = pf32[:, 258:266]
            idf = pf32[:, 266:394]
            wcol = pf32[:, 394:426]  # per-partition conv taps, col = ct*4+j
            idbf = pbf[:, 256:384]
            maskSI2 = pbf[:, 384:896]
            ones_col = consts.tile([128, 1], bf16, tag="ones_col")
            nc.vector.memset(ones_col, 1.0)
            ones2 = consts.tile([2, 128], bf16, tag="ones2")
            nc.vector.memset(ones2, 1.0)
            ones1 = consts.tile([1, 128], bf16, tag="ones1")
            nc.vector.memset(ones1, 1.0)
            zeros_s = consts.tile([128, 128], f32, tag="zeros_s")
            nc.vector.memset(zeros_s, 0.0)
            epsc = consts.tile([128, 1], f32, tag="epsc")
            nc.vector.memset(epsc, 1e-6)

            # ---------------- G math part A (only needs pf32) -------------
            ea_t = consts.tile([128, 128], f32, tag="ea_t")
            nc.scalar.activation(ea_t, a_pk, AF.Exp, bias=dtb, scale=1.0)
            spa = consts.tile([128, 128], f32, tag="spa")
            nc.scalar.activation(spa, ea_t, AF.Ln, bias=1.0, scale=1.0)
            g_pk = consts.tile([128, 128], f32, tag="g_pk")
            nc.vector.tensor_scalar_mul(g_pk, spa, negea)
            G = consts.tile([128, 128], f32, tag="G")
            nc.vector.tensor_tensor_scan(
                G, g_pk, zeros_s, 0.0, op0=OP.add, op1=OP.add)
            eb_t = consts.tile([128, 128], f32, tag="eb_t")
            nc.scalar.activation(eb_t, b_pk, AF.Exp, scale=-1.0)
            spnb = consts.tile([128, 128], f32, tag="spnb")  # softplus(-b) = -ln(beta)
            nc.scalar.activation(spnb, eb_t, AF.Ln, bias=1.0, scale=1.0)
            # beta = sigmoid(b) = 1/(1+exp(-b)) on DVE (keeps ACT on one LUT)
            ebp1 = scr.tile([128, 128], f32, tag="ebp1")
            nc.vector.tensor_scalar_add(ebp1, eb_t, 1.0)
            beta_pk = consts.tile([128, 128], f32, tag="beta_pk")
            nc.vector.reciprocal(beta_pk, ebp1)

            # ---------------- conv + silu + norms ----------------
            # ykq[kh]: [128, 8192] interleaved per (b, chunk): [k(128) | q(128)]
            # ytv[hl]: [128, 4096] plain
            ykq = [ypool.tile([128, B * NCH * 256], bf16, tag=f"ykq{kh}",
                              name=f"ykq{kh}") for kh in range(2)]
            ytv = [ypool.tile([128, B * T], bf16, tag=f"ytv{hl}",
                              name=f"ytv{hl}") for hl in range(4)]

            pset_stack = contextlib.ExitStack()
            pset = pset_stack.enter_context(
                tc.tile_pool(name="pset", bufs=1, space="PSUM"))
            # norm accumulator: cols 0-63 = k (j), 64-127 = q (64+j)
            nacc = pset.tile([128, 128], f32, tag="nacc")

            def conv_ct_dve(xin, ct):
                # depthwise conv on DVE via per-partition tensor_scalar
                # chains (v heads only; PE stays on k/q + chunk work)
                xs = xin.tile([128, B * TPAD], bf16, tag="xs", name="xs")
                nc.sync.dma_start(xs, x_cm[ct])
                for b_ in range(B):
                    for blk in range(4):
                        base = b_ * TPAD + blk * 512
                        acc = xin.tile([128, 512], bf16, tag="vacc",
                                       bufs=8, name="acc")
                        nc.vector.tensor_scalar_mul(
                            acc, xs[:, base: base + 512],
                            wcol[:, ct * 4: ct * 4 + 1])
                        for j in range(1, KW):
                            acc2 = xin.tile([128, 512], bf16, tag="vacc",
                                            bufs=8, name="acc2")
                            nc.vector.scalar_tensor_tensor(
                                acc2, xs[:, base + j: base + j + 512],
                                wcol[:, ct * 4 + j: ct * 4 + j + 1], acc,
                                op0=OP.mult, op1=OP.add)
                            acc = acc2
                        ydst = ytv[ct - 4][:, b_ * T + blk * 512:
                                           b_ * T + (blk + 1) * 512]
                        if sim_compat:
                            zc = xin.tile([128, 512], bf16, tag="zc",
                                          name="zc")
                            nc.scalar.activation(
                                zc, acc, AF.Identity,
                                bias=cbt[:, ct:ct + 1], scale=1.0)
                            sg = xin.tile([128, 512], bf16, tag="sg",
                                          name="sg")
                            nc.scalar.activation(sg, zc, AF.Sigmoid)
                            nc.vector.tensor_mul(ydst, zc, sg)
                        else:
                            nc.scalar.activation(
                                ydst, acc, AF.Silu,
                                bias=cbt[:, ct:ct + 1], scale=1.0)

            def conv_ct(xin, pcvp, ct):
                xs = xin.tile([128, B * TPAD], bf16, tag="xs", name="xs")
                nc.sync.dma_start(xs, x_cm[ct])
                for b_ in range(B):
                    for bp in range(2):
                        pcv = pcvp.tile([128, 1024], f32, tag="pconv",
                                        name="pcv")
                        for h in range(2):
                            base = b_ * TPAD + bp * 1024 + h * 512
                            for j in range(KW):
                                nc.tensor.matmul(
                                    pcv[:, h * 512:(h + 1) * 512],
                                    wd[:, ct, j, :],
                                    xs[:, base + j: base + j + 512],
                                    start=(j == 0), stop=(j == KW - 1))
                        if ct >= 4:
                            dst = ytv[ct - 4][:, b_ * T + bp * 1024:
                                              b_ * T + (bp + 1) * 1024]
                            src = pcv
                        else:
                            kh = ct - 2 if ct >= 2 else ct
                            half = 0 if ct >= 2 else 1  # k -> 0, q -> 1
                            seg = ykq[kh][:, (b_ * 16 + 8 * bp) * 256:
                                          (b_ * 16 + 8 * bp + 8) * 256]
                            dst = seg.rearrange(
                                "p (c two t) -> p c two t", c=8, two=2,
                                t=128)[:, :, half:half + 1, :]
                            src = pcv.rearrange(
                                "p (c o t) -> p c o t", c=8, o=1, t=128)
                        if sim_compat:
                            zc = xin.tile([128, 1024], bf16, tag="zc",
                                          name="zc")
                            nc.scalar.activation(
                                zc, pcv, AF.Identity,
                                bias=cbt[:, ct:ct + 1], scale=1.0)
                            sg = xin.tile([128, 1024], bf16, tag="sg",
                                          name="sg")
                            nc.scalar.activation(sg, zc, AF.Sigmoid)
                            nc.vector.tensor_mul(
                                dst, zc.rearrange("p (c o t) -> p c o t",
                                                  c=8, o=1, t=128)
                                if ct < 4 else zc,
                                sg.rearrange("p (c o t) -> p c o t",
                                             c=8, o=1, t=128)
                                if ct < 4 else sg)
                        else:
                            nc.scalar.activation(
                                dst, src, AF.Silu,
                                bias=cbt[:, ct:ct + 1], scale=1.0)

            def norms_kh(ysqp, kh):
                # squares + N=1 ones-matmul column sums into kacc/qacc psum
                for b_ in range(B):
                    ysq = ysqp.tile([128, 4096], bf16, tag="ysq", name="ysq")
                    nc.scalar.activation(
                        ysq, ykq[kh][:, b_ * 4096:(b_ + 1) * 4096], AF.Square)
                    for c_ in range(NCH):
                        j = b_ * 32 + kh * 16 + c_
                        nc.tensor.matmul(
                            nacc[:, j:j + 1],
                            ysq[:, c_ * 256: c_ * 256 + 128], ones_col,
                            start=True, stop=True)
                        nc.tensor.matmul(
                            nacc[:, 64 + j:64 + j + 1],
                            ysq[:, c_ * 256 + 128: c_ * 256 + 256], ones_col,
                            start=True, stop=True)

            with tc.tile_pool(name="xin", bufs=2) as xin, \
                    tc.tile_pool(name="ysqp", bufs=2) as ysqp, \
                    tc.tile_pool(name="pcv", bufs=2, space="PSUM") as pcvp:
                for ct in (2, 0):
                    conv_ct(xin, pcvp, ct)
                norms_kh(ysqp, 0)
                for ct in (3, 1):
                    conv_ct(xin, pcvp, ct)
                norms_kh(ysqp, 1)
                for ct in (4, 5, 6, 7):
                    conv_ct(xin, pcvp, ct)

            # ---------------- ln(norms) + packed expand ----------------
            lnkq = scr.tile([128, 128], f32, tag="lnkq")
            nc.scalar.activation(lnkq, nacc, AF.Ln, bias=epsc, scale=1.0)
            # nacc layout [t, col=b*32+kh*16+c (+64 for q)] -> transpose,
            # then expand dups to packed [row=inst*16+c, t]
            Lk_pk = consts.tile([128, 128], f32, tag="Lk_pk")
            Lq_pk = consts.tile([128, 128], f32, tag="Lq_pk")
            pt = pset.tile([128, 128], f32, tag="ptr", bufs=2)
            nc.tensor.transpose(pt, lnkq, idf)
            st = scr.tile([128, 128], f32, tag="snorm")
            nc.vector.tensor_copy(st, pt)
            for qoff, dst in ((0, Lk_pk), (64, Lq_pk)):
                for b_ in range(B):
                    for kh in range(2):
                        for dup in range(2):
                            inst = b_ * 4 + kh * 2 + dup
                            nc.sync.dma_start(
                                dst[inst * 16: inst * 16 + 16, :],
                                st[qoff + b_ * 32 + kh * 16:
                                   qoff + b_ * 32 + kh * 16 + 16, :])

            # ---------------- G math part B (needs norms) ----------------
            t1 = scr.tile([128, 128], f32, tag="t1")     # G - 0.5*Lk
            nc.vector.scalar_tensor_tensor(
                t1, Lk_pk, -0.5, G, op0=OP.mult, op1=OP.add)
            Gt = consts.tile([128, 128], f32, tag="Gt")  # G + ln(beta) - 0.5*Lk
            nc.vector.tensor_sub(Gt, t1, spnb)
            Gk = consts.tile([128, 128], f32, tag="Gk")  # G + 0.5*Lk
            nc.vector.scalar_tensor_tensor(
                Gk, Lk_pk, 0.5, G, op0=OP.mult, op1=OP.add)
            t2 = scr.tile([128, 128], f32, tag="t2")
            nc.vector.scalar_tensor_tensor(
                t2, Lq_pk, -0.5, G, op0=OP.mult, op1=OP.add)
            Gq = consts.tile([128, 128], f32, tag="Gq")
            nc.vector.tensor_scalar_add(Gq, t2, -LN_HALF_DK)
            expGt = scr.tile([128, 128], f32, tag="expGt")
            nc.scalar.activation(expGt, Gt, AF.Exp)
            negexpGt = consts.tile([128, 128], f32, tag="negexpGt")
            nc.vector.tensor_scalar_mul(negexpGt, expGt, -1.0)
            decrn = consts.tile([128, 128], f32, tag="decrn")  # exp(G_C - Gk)
            nc.scalar.activation(decrn, Gk, AF.Exp, bias=G[:, 127:128],
                                 scale=-1.0)
            eGqn = consts.tile([128, 128], f32, tag="eGqn")    # exp(Gq)
            nc.scalar.activation(eGqn, Gq, AF.Exp)

            # eGC broadcast [128, 128] (col r = exp(G_C(r)) replicated)
            eGCc = scr.tile([128, 1], bf16, tag="eGCc")
            nc.scalar.activation(eGCc, G[:, 127:128], AF.Exp)
            pt1 = pset.tile([1, 128], bf16, tag="ptr", bufs=2)
            nc.tensor.transpose(pt1, eGCc, idbf)
            eGCrow = scr.tile([1, 128], bf16, tag="eGCrow")
            nc.vector.tensor_copy(eGCrow, pt1)
            pb = pset.tile([128, 128], f32, tag="ptr", bufs=2)
            nc.tensor.matmul(pb, ones1, eGCrow, start=True, stop=True)
            eGCb = consts.tile([128, 128], f32, tag="eGCb")
            nc.vector.tensor_copy(eGCb, pb)

            # transposed per-time tables: [128(t), 128(col=r)]
            beta_T = consts.tile([128, 128], f32, tag="beta_T")
            negeGt_T = consts.tile([128, 128], f32, tag="negeGt_T")
            decrn_T = consts.tile([128, 128], f32, tag="decrn_T")
            eGqn_T = consts.tile([128, 128], f32, tag="eGqn_T")
            negGk_T = consts.tile([128, 128], f32, tag="negGk_T")
            for src_t, dst, sc in ((beta_pk, beta_T, 1.0),
                                   (negexpGt, negeGt_T, 1.0),
                                   (decrn, decrn_T, 1.0),
                                   (eGqn, eGqn_T, 1.0),
                                   (Gk, negGk_T, -1.0)):
                pt2 = pset.tile([128, 128], f32, tag="ptr", bufs=2)
                nc.tensor.transpose(pt2, src_t, idf)
                if sc == 1.0:
                    nc.vector.tensor_copy(dst, pt2)
                else:
                    nc.vector.tensor_scalar_mul(dst, pt2, sc)

            # hi/lo bf16 splits of Gt/Gq
            def hilo(src_t, nm):
                hi = consts.tile([128, 128], bf16, name=f"{nm}h", tag=f"{nm}h")
                nc.vector.tensor_copy(hi, src_t)
                lo = consts.tile([128, 128], bf16, name=f"{nm}l", tag=f"{nm}l")
                nc.vector.tensor_sub(lo, src_t, hi)
                return hi, lo

            Gth, Gtl = hilo(Gt, "Gt")
            Gqh, Gql = hilo(Gq, "Gq")

            # flatten hi/lo tables to [2(hi,lo), (c,i,w,t)] in SBUF so the
            # per-chunk per-pair rank-1 matmul rhs is one CONTIGUOUS
            # [2, 512] slice ([Gt_i0|Gq_i0|Gt_i1|Gq_i1]).  Own pool
            # (opened after conv scratch closed) so the stack allocator
            # reuses the freed conv region.
            flatp = ctx.enter_context(tc.tile_pool(name="flatp", bufs=1))
            GtGq_flat = flatp.tile([2, NCH * 8 * 2 * 128], bf16,
                                   tag="GtGq_flat", name="GtGq_flat")
            fl_v = GtGq_flat.rearrange("p (c i w t) -> p c i w t",
                                       c=NCH, i=8, w=2)
            for row, w, src_t in ((0, 0, Gth), (1, 0, Gtl),
                                  (0, 1, Gqh), (1, 1, Gql)):
                for i in range(8):
                    nc.sync.dma_start(
                        fl_v[row:row + 1, :, i:i + 1, w:w + 1, :],
                        src_t[i * 16:(i + 1) * 16, :])

            # ---------------- state init ----------------
            S = {}
            for p in range(4):
                s0 = consts.tile([128, 256], bf16, name=f"S{p}", tag=f"S{p}")
                nc.vector.memset(s0, 0.0)
                S[p] = s0

            pset_stack.close()

            # ---------------- chunk loop ----------------
            ps = ctx.enter_context(tc.tile_pool(name="ps", bufs=1,
                                                space="PSUM"))

            def rof(inst, c_):
                return inst * 16 + c_

            for c_ in range(NCH):
                def kqslice(kh, b_, width, off=0):
                    base = (b_ * 16 + c_) * 256 + off
                    return ykq[kh][:, base: base + width]

                # P1: decay psums [L0|A0|L1|A1] + E2
                pDs = []
                for p, (b_, kh) in enumerate(pairs):
                    i0 = b_ * 4 + kh * 2
                    pD = ps.tile([128, 512], f32, tag="pD", bufs=2,
                                 name="pD")
                    nc.tensor.matmul(pD, idbf, maskSI2,
                                     start=True, stop=False)
                    nc.tensor.matmul(
                        pD, ones2,
                        GtGq_flat[:, c_ * 2048 + i0 * 256:
                                  c_ * 2048 + (i0 + 2) * 256],
                        start=False, stop=True)
                    pDs.append(pD)
                E2 = {}
                for p, (b_, kh) in enumerate(pairs):
                    for dup in range(2):
                        inst = b_ * 4 + kh * 2 + dup
                        r = rof(inst, c_)
                        e2 = work.tile([128, 256], bf16, tag="e2", bufs=8,
                                       name="e2")
                        nc.scalar.activation(
                            e2, pDs[p][:, dup * 256:(dup + 1) * 256],
                            AF.Exp, bias=negGk_T[:, r:r + 1], scale=1.0)
                        E2[inst] = e2

                # P2: KKQ + pR psums (packed per-pair bank) + KKQ evac
                # pX layout per use:
                #   pkr:  pk [0:256], pR [256:512]
                #   pU:   pU1_d0 [0:128], pU1_d1 [128:256],
                #         pU2_d0 [256:384], pU2_d1 [384:512]
                #   pout: pQS [0:256], pAU_d0 [256:384], pAU_d1 [384:512]
                #   pS:   4 insts x [128]
                kq_sb = {}
                pkr = []
                for p, (b_, kh) in enumerate(pairs):
                    px = ps.tile([128, 512], f32, tag="pX", bufs=4,
                                 name="pkr")
                    pkr.append(px)
                    nc.tensor.matmul(px[:, 0:256], kqslice(kh, b_, 128),
                                     kqslice(kh, b_, 256),
                                     start=True, stop=True)
                    # negated evac so nl is a plain tensor_tensor multiply
                    kq = work.tile([128, 256], bf16, tag="kq", bufs=4,
                                   name="kq")
                    nc.scalar.mul(kq, px[:, 0:256], -1.0)
                    kq_sb[p] = kq

                # P3: nl = E2 * (-[KK|KQ])  (bf16 2x DVE)
                nL = {}
                for p, (b_, kh) in enumerate(pairs):
                    for dup in range(2):
                        inst = b_ * 4 + kh * 2 + dup
                        nl = work.tile([128, 256], bf16, tag="nl", bufs=8,
                                       name="nl")
                        nc.vector.tensor_mul(nl, E2[inst], kq_sb[p])
                        nL[inst] = nl

                # P4: k + v transposes (packed bf16 bank) + ktl/bv (ACT)
                # pTa: pkt p0..p3 at [p*128], pVT inst0..3 at [512+hl*128]
                # pTb: pVT inst4..7 at [hl*128]
                pTa = ps.tile([128, 1024], bf16, tag="pT", bufs=2,
                              name="pTa")
                pTb = ps.tile([128, 1024], bf16, tag="pT", bufs=2,
                              name="pTb")
                ktl = {}
                for p, (b_, kh) in enumerate(pairs):
                    pkt = pTa[:, p * 128:(p + 1) * 128]
                    nc.tensor.transpose(pkt, kqslice(kh, b_, 128), idbf)
                    for dup in range(2):
                        inst = b_ * 4 + kh * 2 + dup
                        r = rof(inst, c_)
                        kt = work.tile([128, 128], bf16, tag="ktl", bufs=8,
                                       name="kt")
                        nc.scalar.mul(kt, pkt, decrn_T[:, r:r + 1])
                        ktl[inst] = kt

                bv = {}
                for b_ in range(B):
                    for hl in range(4):
                        inst = b_ * 4 + hl
                        r = rof(inst, c_)
                        if b_ == 0:
                            pVT = pTa[:, 512 + hl * 128: 640 + hl * 128]
                        else:
                            pVT = pTb[:, hl * 128:(hl + 1) * 128]
                        nc.tensor.transpose(
                            pVT, ytv[hl][:, b_ * T + c_ * C:
                                         b_ * T + (c_ + 1) * C], idbf)
                        b1 = work.tile([128, 128], bf16, tag="bv", bufs=8,
                                       name="b1")
                        nc.scalar.mul(b1, pVT, beta_T[:, r:r + 1])
                        bv[inst] = b1

                # P6: pR (pair, into pkr bank) + R (DVE, pair tile halves)
                for p, (b_, kh) in enumerate(pairs):
                    nc.tensor.matmul(pkr[p][:, 256:512],
                                     kqslice(kh, b_, 128), S[p],
                                     start=True, stop=True)
                R = {}
                Rp = {}
                for p, (b_, kh) in enumerate(pairs):
                    Rpair = work.tile([128, 256], bf16, tag="R", bufs=8,
                                      name="Rpair")
                    Rp[p] = Rpair
                    for dup in range(2):
                        inst = b_ * 4 + kh * 2 + dup
                        r = rof(inst, c_)
                        nc.vector.scalar_tensor_tensor(
                            Rpair[:, dup * 128:(dup + 1) * 128],
                            pkr[p][:, 256 + dup * 128:
                                   256 + (dup + 1) * 128],
                            negeGt_T[:, r:r + 1], bv[inst],
                            op0=OP.mult, op1=OP.add)
                        R[inst] = Rpair[:, dup * 128:(dup + 1) * 128]

                # P7/P8: Horner — U1 = nL@R + R, U2 = nL@U1 + R
                # (pair-wide DVE stt evacuations)
                pUs = []
                U1 = {}
                for p, (b_, kh) in enumerate(pairs):
                    pU = ps.tile([128, 512], f32, tag="pX", bufs=4,
                                 name="pU")
                    pUs.append(pU)
                    for dup in range(2):
                        inst = b_ * 4 + kh * 2 + dup
                        nc.tensor.matmul(pU[:, dup * 128:(dup + 1) * 128],
                                         nL[inst][:, 0:128], R[inst],
                                         start=True, stop=True)
                    u1p = work.tile([128, 256], bf16, tag="U1", bufs=8,
                                    name="u1p")
                    nc.vector.scalar_tensor_tensor(
                        u1p, pU[:, 0:256], 1.0, Rp[p],
                        op0=OP.mult, op1=OP.add)
                    for dup in range(2):
                        inst = b_ * 4 + kh * 2 + dup
                        U1[inst] = u1p[:, dup * 128:(dup + 1) * 128]
                U2 = {}
                for p, (b_, kh) in enumerate(pairs):
                    for dup in range(2):
                        inst = b_ * 4 + kh * 2 + dup
                        nc.tensor.matmul(
                            pUs[p][:, 256 + dup * 128: 256 + (dup + 1) * 128],
                            nL[inst][:, 0:128], U1[inst],
                            start=True, stop=True)
                    u2p = work.tile([128, 256], bf16, tag="U2", bufs=8,
                                    name="u2p")
                    nc.vector.scalar_tensor_tensor(
                        u2p, pUs[p][:, 256:512], 1.0, Rp[p],
                        op0=OP.mult, op1=OP.add)
                    for dup in range(2):
                        inst = b_ * 4 + kh * 2 + dup
                        U2[inst] = u2p[:, dup * 128:(dup + 1) * 128]

                # P9: output O = eGqn*(q S) - (-A)U  (pout bank per pair)
                pouts = []
                for p, (b_, kh) in enumerate(pairs):
                    po = ps.tile([128, 512], f32, tag="pX", bufs=4,
                                 name="pout")
                    pouts.append(po)
                    nc.tensor.matmul(po[:, 0:256],
                                     kqslice(kh, b_, 128, off=128),
                                     S[p], start=True, stop=True)
                    for dup in range(2):
                        inst = b_ * 4 + kh * 2 + dup
                        nc.tensor.matmul(
                            po[:, 256 + dup * 128: 256 + (dup + 1) * 128],
                            nL[inst][:, 128:256], U2[inst],
                            start=True, stop=True)
                for p, (b_, kh) in enumerate(pairs):
                    aup = work.tile([128, 256], bf16, tag="AU", bufs=4,
                                    name="aup")
                    nc.scalar.copy(aup, pouts[p][:, 256:512])
                    for dup in range(2):
                        hl = kh * 2 + dup
                        inst = b_ * 4 + hl
                        r = rof(inst, c_)
                        O_sb = work.tile([128, 128], bf16, tag="O", bufs=4,
                                         name="O_sb")
                        nc.vector.scalar_tensor_tensor(
                            O_sb, pouts[p][:, dup * 128:(dup + 1) * 128],
                            eGqn_T[:, r:r + 1],
                            aup[:, dup * 128:(dup + 1) * 128],
                            op0=OP.mult, op1=OP.subtract)
                        nc.sync.dma_start(
                            out_d[b_, c_ * C:(c_ + 1) * C,
                                  hl * DV:(hl + 1) * DV], O_sb)

                # P10: state update (pS packed 4-per-bank, 2 banks/chunk)
                Snew = {}
                for p, (b_, kh) in enumerate(pairs):
                    Snew[p] = spool.tile([128, 256], bf16, tag=f"Sn{p}",
                                         name="Snew")
                for half in range(2):
                    pSb = ps.tile([128, 512], f32, tag="pX", bufs=4,
                                  name="pSb")
                    for pi in range(2):
                        p = half * 2 + pi
                        b_, kh = pairs[p]
                        for dup in range(2):
                            inst = b_ * 4 + kh * 2 + dup
                            r = rof(inst, c_)
                            sl = pSb[:, (pi * 2 + dup) * 128:
                                     (pi * 2 + dup + 1) * 128]
                            nc.tensor.matmul(sl, ktl[inst], U2[inst],
                                             start=True, stop=True)
                            nc.vector.scalar_tensor_tensor(
                                Snew[p][:, dup * 128:(dup + 1) * 128],
                                S[p][:, dup * 128:(dup + 1) * 128],
                                eGCb[:, r:r + 1], sl,
                                op0=OP.mult, op1=OP.add)
                for p in range(4):
                    S[p] = Snew[p]
    nc.compile()
    return nc


# ---------------------------------------------------------------------------
# host-side prep
# ---------------------------------------------------------------------------

def host_inputs(core, mixed_qkv, a, b, conv_weight, conv_bias, A_log, dt_bias):
    import ml_dtypes
    bf = ml_dtypes.bfloat16
    f32 = np.float32

    chsel = np.r_[256 * core: 256 * core + 256,
                  2048 + 256 * core: 2048 + 256 * core + 256,
                  4096 + 512 * core: 4096 + 512 * core + 512]
    xs = mixed_qkv[:, :, chsel]                     # [B,T,1024]
    ws = conv_weight[chsel].astype(f32)             # [1024,4]
    cbs = conv_bias[chsel].astype(f32)

    x_cm = np.zeros((8, 128, B * TPAD), dtype=bf)
    xt = np.ascontiguousarray(xs.transpose(2, 0, 1))  # [1024,B,T]
    for b_ in range(B):
        x_cm[:, :, b_ * TPAD + 3: (b_ + 1) * TPAD] = \
            xt[:, b_, :].reshape(8, 128, T).astype(bf)

    wdiag = np.zeros((8, KW, 128, 128), dtype=bf)
    wr = ws.reshape(8, 128, KW)
    di = np.arange(128)
    for t_ in range(8):
        for j in range(KW):
            wdiag[t_, j, di, di] = wr[t_, :, j].astype(bf)

    cb_t = np.ascontiguousarray(cbs.reshape(8, 128).T).astype(f32)  # [128,8]

    a_pk = np.zeros((128, 128), f32)
    b_pk = np.zeros((128, 128), f32)
    dtb = np.zeros((128, 1), f32)
    negea = np.zeros((128, 1), f32)
    for b_ in range(B):
        for hl in range(4):
            hg = 4 * core + hl
            inst = b_ * 4 + hl
            a_pk[inst * 16: (inst + 1) * 16] = a[b_, :, hg].reshape(NCH, C)
            b_pk[inst * 16: (inst + 1) * 16] = b[b_, :, hg].reshape(NCH, C)
            dtb[inst * 16: (inst + 1) * 16] = dt_bias[hg]
            negea[inst * 16: (inst + 1) * 16] = -np.exp(A_log[hg])

    idx = np.arange(C)
    maskS = np.where(idx[:, None] < idx[None, :], 0.0, -1e6).astype(bf)
    maskI = np.where(idx[:, None] <= idx[None, :], 0.0, -1e6).astype(bf)
    idbf = np.eye(128, dtype=bf)
    idf32 = np.eye(128, dtype=f32)

    wcol_t = np.ascontiguousarray(
        wr.transpose(1, 0, 2).reshape(128, 32)).astype(f32)  # [p, ct*4+j]
    pf32 = np.concatenate(
        [a_pk, b_pk, dtb, negea, cb_t, idf32, wcol_t], axis=1).astype(f32)
    # maskSI2 ordered [S|I|S|I] to match the pD layout [L0|A0|L1|A1]
    pbf = np.ascontiguousarray(np.concatenate(
        [maskS, maskI, idbf, maskS, maskI, maskS, maskI], axis=1)).astype(bf)
    return {"x_cm": x_cm, "wdiag": wdiag, "pf32": pf32, "pbf": pbf}


_CACHED = {}


def _get_program():
    if "nc" not in _CACHED:
        _CACHED["nc"] = build_program()
    return _CACHED["nc"]


def kernel(mixed_qkv, a, b, conv_weight, conv_bias, A_log, dt_bias, trace=False):
    f32 = np.float32
    mixed_qkv = np.asarray(mixed_qkv, f32)
    a = np.asarray(a, f32)
    b = np.asarray(b, f32)
    conv_weight = np.asarray(conv_weight, f32)
    conv_bias = np.asarray(conv_bias, f32)
    A_log = np.asarray(A_log, f32)
    dt_bias = np.asarray(dt_bias, f32)

    from concourse import bass_utils

    nc = _get_program()
    in_maps = [host_inputs(core, mixed_qkv, a, b, conv_weight, conv_bias,
                           A_log, dt_bias) for core in range(NCORE)]
    res = bass_utils.run_bass_kernel_spmd(
        nc, in_maps, core_ids=list(range(NCORE)), trace=trace)

    out = np.empty((B, T, HV * DV), f32)
    for core in range(NCORE):
        out[:, :, 512 * core: 512 * (core + 1)] = \
            res.results[core]["out"].astype(f32)
    if trace:
        return out, res
    return out


# revision 44
# speedup vs baseline: 1.2149x; 1.0897x over previous
"""Gated DeltaNet (Qwen3.5-style) forward on 8 Trainium2 NeuronCores.

Sharding: tensor-parallel over heads. Core i owns v-heads 4i..4i+3 and
k-heads 2i..2i+1 (GVA rep=2), both batch rows. Each core runs an identical
Bass program (SPMD) on its head-slice; no collectives.

Device algorithm (per core):
  depthwise causal conv(K=4) on PE (diagonal-weight matmuls) + SiLU on ACT
  -> chunked delta rule with chunk C=128: within-chunk unit-lower-triangular
  solve (I+L)^-1 via 3-term Neumann/Horner, inter-chunk recurrence on
  S[128,256] per (batch,kh) pair (two v-heads side by side). q/k l2-norms
  are folded into the decay exponents in log space.

v2 restructure vs baseline:
  - k|q interleaved conv output layout -> KK|KQ in one N=256 matmul/pair
  - decay row-term exp(-Gk(s)) applied via ACT-Exp bias column, so the
    per-chunk exponent tables need only a rank-1 [Gt|Gq] matmul (lhsT =
    static ones2); per-chunk table fetch is one contiguous DMA from a
    DRAM-staged gather built once at setup
  - q/k norms via N=1 ones-matmuls on squared tiles (no PE transposes)
  - pair-merged S[128,256]: pR/pQS are N=256 matmuls
  - Horner iter1 accumulates +R on PE (idbf matmul) and evacuates on ACT
    to balance DVE/ACT load
"""

import numpy as np

B, T = 2, 2048
HK, HV, DK, DV = 16, 32, 128, 128
CONV_DIM = 2 * HK * DK + HV * DV
KW = 4            # conv taps
C = 128           # chunk length
NCH = T // C      # 16 chunks
TPAD = T + 3      # left zero-pad per batch segment
NCORE = 8
LN_HALF_DK = 0.5 * float(np.log(DK))


# ---------------------------------------------------------------------------
# device program
# ---------------------------------------------------------------------------

def build_program(sim_compat=False):
    import concourse.bacc as bacc
    import concourse.tile as tile
    from concourse import mybir

    f32 = mybir.dt.float32
    bf16 = mybir.dt.bfloat16
    AF = mybir.ActivationFunctionType
    OP = mybir.AluOpType

    nc = bacc.Bacc("TRN2", target_bir_lowering=False, debug=False)

    dram = {}
    def din(name, shape, dt):
        dram[name] = nc.dram_tensor(name, shape, dt, kind="ExternalInput").ap()
        return dram[name]

    x_cm = din("x_cm", [8, 128, B * TPAD], bf16)
    wdiag = din("wdiag", [8, KW, 128, 128], bf16)
    # pf32 = [a_pk | b_pk | dtb | negea | cb | idf32 | wcol | kapb | ekapb]
    pf32_in = din("pf32", [128, 682], f32)
    # pbf = [maskS | maskI | idbf | maskSI2 | mask01SI]
    pbf_in = din("pbf", [128, 1152], bf16)
    out_d = nc.dram_tensor("out", [B, T, 4 * DV], bf16, kind="ExternalOutput").ap()

    pairs = [(b_, kh) for b_ in range(B) for kh in range(2)]

    with tile.TileContext(nc) as tc:
        import contextlib
        ctx = contextlib.ExitStack()
        with ctx:
            consts = ctx.enter_context(tc.tile_pool(name="consts", bufs=1))
            ypool = ctx.enter_context(tc.tile_pool(name="ypool", bufs=1))
            spool = ctx.enter_context(tc.tile_pool(name="spool", bufs=2))
            scr = ctx.enter_context(tc.tile_pool(name="scr", bufs=4))
            work = ctx.enter_context(tc.tile_pool(name="work", bufs=3))
            dscr = ctx.enter_context(
                tc.tile_pool(name="dscr", bufs=1, space="DRAM"))

            # ---------------- constants in ----------------
            pf32 = consts.tile([128, 682], f32, tag="pf32")
            nc.sync.dma_start(pf32, pf32_in)
            pbf = consts.tile([128, 1152], bf16, tag="pbf")
            nc.sync.dma_start(pbf, pbf_in)
            wd = consts.tile([128, 8, KW, 128], bf16, tag="wd")
            nc.sync.dma_start(wd, wdiag.rearrange("t j p c -> p t j c"))
            a_pk = pf32[:, 0:128]
            b_pk = pf32[:, 128:256]
            dtb = pf32[:, 256:257]
            negea = pf32[:, 257:258]
            cbt = pf32[:, 258:266]
            idf = pf32[:, 266:394]
            wcol = pf32[:, 394:426]  # per-partition conv taps, col = ct*4+j
            kapb = pf32[:, 426:554]   # exponent shift kappa(r), bcast rows
            ekapb = pf32[:, 554:682]  # exp(+kappa(r)), bcast rows
            idbf = pbf[:, 256:384]
            maskSI2 = pbf[:, 384:896]
            mask01 = pbf[:, 896:1152]  # 0/1 [maskS01 | maskI01]
            ones_col = consts.tile([128, 1], bf16, tag="ones_col")
            nc.vector.memset(ones_col, 1.0)
            ones2 = consts.tile([2, 128], bf16, tag="ones2")
            nc.vector.memset(ones2, 1.0)
            ones1 = consts.tile([1, 128], bf16, tag="ones1")
            nc.vector.memset(ones1, 1.0)
            zeros_s = consts.tile([128, 128], f32, tag="zeros_s")
            nc.vector.memset(zeros_s, 0.0)
            epsc = consts.tile([128, 1], f32, tag="epsc")
            nc.vector.memset(epsc, 1e-6)

            # ---------------- G math part A (only needs pf32) -------------
            ea_t = consts.tile([128, 128], f32, tag="ea_t")
            nc.scalar.activation(ea_t, a_pk, AF.Exp, bias=dtb, scale=1.0)
            spa = consts.tile([128, 128], f32, tag="spa")
            nc.scalar.activation(spa, ea_t, AF.Ln, bias=1.0, scale=1.0)
            g_pk = consts.tile([128, 128], f32, tag="g_pk")
            nc.vector.tensor_scalar_mul(g_pk, spa, negea)
            G = consts.tile([128, 128], f32, tag="G")
            nc.vector.tensor_tensor_scan(
                G, g_pk, zeros_s, 0.0, op0=OP.add, op1=OP.add)
            eb_t = consts.tile([128, 128], f32, tag="eb_t")
            nc.scalar.activation(eb_t, b_pk, AF.Exp, scale=-1.0)
            spnb = consts.tile([128, 128], f32, tag="spnb")  # softplus(-b) = -ln(beta)
            nc.scalar.activation(spnb, eb_t, AF.Ln, bias=1.0, scale=1.0)
            # beta = sigmoid(b) = 1/(1+exp(-b)) on DVE (keeps ACT on one LUT)
            ebp1 = scr.tile([128, 128], f32, tag="ebp1")
            nc.vector.tensor_scalar_add(ebp1, eb_t, 1.0)
            beta_pk = consts.tile([128, 128], f32, tag="beta_pk")
            nc.vector.reciprocal(beta_pk, ebp1)

            # ---------------- conv + silu + norms ----------------
            # ykq[kh]: [128, 8192] interleaved per (b, chunk): [k(128) | q(128)]
            # ytv[hl]: [128, 4096] plain
            ykq = [ypool.tile([128, B * NCH * 256], bf16, tag=f"ykq{kh}",
                              name=f"ykq{kh}") for kh in range(2)]
            ytv = [ypool.tile([128, B * T], bf16, tag=f"ytv{hl}",
                              name=f"ytv{hl}") for hl in range(4)]

            pset_stack = contextlib.ExitStack()
            pset = pset_stack.enter_context(
                tc.tile_pool(name="pset", bufs=1, space="PSUM"))
            # norm accumulator: cols 0-63 = k (j), 64-127 = q (64+j)
            nacc = pset.tile([128, 128], f32, tag="nacc")

            def conv_ct_dve(xin, ct):
                # depthwise conv on DVE via per-partition tensor_scalar
                # chains (v heads only; PE stays on k/q + chunk work)
                xs = xin.tile([128, B * TPAD], bf16, tag="xs", name="xs")
                nc.sync.dma_start(xs, x_cm[ct])
                for b_ in range(B):
                    for blk in range(4):
                        base = b_ * TPAD + blk * 512
                        acc = xin.tile([128, 512], bf16, tag="vacc",
                                       bufs=8, name="acc")
                        nc.vector.tensor_scalar_mul(
                            acc, xs[:, base: base + 512],
                            wcol[:, ct * 4: ct * 4 + 1])
                        for j in range(1, KW):
                            acc2 = xin.tile([128, 512], bf16, tag="vacc",
                                            bufs=8, name="acc2")
                            nc.vector.scalar_tensor_tensor(
                                acc2, xs[:, base + j: base + j + 512],
                                wcol[:, ct * 4 + j: ct * 4 + j + 1], acc,
                                op0=OP.mult, op1=OP.add)
                            acc = acc2
                        ydst = ytv[ct - 4][:, b_ * T + blk * 512:
                                           b_ * T + (blk + 1) * 512]
                        if sim_compat:
                            zc = xin.tile([128, 512], bf16, tag="zc",
                                          name="zc")
                            nc.scalar.activation(
                                zc, acc, AF.Identity,
                                bias=cbt[:, ct:ct + 1], scale=1.0)
                            sg = xin.tile([128, 512], bf16, tag="sg",
                                          name="sg")
                            nc.scalar.activation(sg, zc, AF.Sigmoid)
                            nc.vector.tensor_mul(ydst, zc, sg)
                        else:
                            nc.scalar.activation(
                                ydst, acc, AF.Silu,
                                bias=cbt[:, ct:ct + 1], scale=1.0)

            def conv_ct(xin, pcvp, ct):
                xs = xin.tile([128, B * TPAD], bf16, tag="xs", name="xs")
                nc.sync.dma_start(xs, x_cm[ct])
                for b_ in range(B):
                    for bp in range(2):
                        pcv = pcvp.tile([128, 1024], f32, tag="pconv",
                                        name="pcv")
                        for h in range(2):
                            base = b_ * TPAD + bp * 1024 + h * 512
                            for j in range(KW):
                                nc.tensor.matmul(
                                    pcv[:, h * 512:(h + 1) * 512],
                                    wd[:, ct, j, :],
                                    xs[:, base + j: base + j + 512],
                                    start=(j == 0), stop=(j == KW - 1))
                        if ct >= 4:
                            dst = ytv[ct - 4][:, b_ * T + bp * 1024:
                                              b_ * T + (bp + 1) * 1024]
                            src = pcv
                        else:
                            kh = ct - 2 if ct >= 2 else ct
                            half = 0 if ct >= 2 else 1  # k -> 0, q -> 1
                            seg = ykq[kh][:, (b_ * 16 + 8 * bp) * 256:
                                          (b_ * 16 + 8 * bp + 8) * 256]
                            dst = seg.rearrange(
                                "p (c two t) -> p c two t", c=8, two=2,
                                t=128)[:, :, half:half + 1, :]
                            src = pcv.rearrange(
                                "p (c o t) -> p c o t", c=8, o=1, t=128)
                        if sim_compat:
                            zc = xin.tile([128, 1024], bf16, tag="zc",
                                          name="zc")
                            nc.scalar.activation(
                                zc, pcv, AF.Identity,
                                bias=cbt[:, ct:ct + 1], scale=1.0)
                            sg = xin.tile([128, 1024], bf16, tag="sg",
                                          name="sg")
                            nc.scalar.activation(sg, zc, AF.Sigmoid)
                            nc.vector.tensor_mul(
                                dst, zc.rearrange("p (c o t) -> p c o t",
                                                  c=8, o=1, t=128)
                                if ct < 4 else zc,
                                sg.rearrange("p (c o t) -> p c o t",
                                             c=8, o=1, t=128)
                                if ct < 4 else sg)
                        else:
                            nc.scalar.activation(
                                dst, src, AF.Silu,
                                bias=cbt[:, ct:ct + 1], scale=1.0)

            def norms_kh(ysqp, kh):
                # squares + N=1 ones-matmul column sums into kacc/qacc psum
                for b_ in range(B):
                    ysq = ysqp.tile([128, 4096], bf16, tag="ysq", name="ysq")
                    nc.scalar.activation(
                        ysq, ykq[kh][:, b_ * 4096:(b_ + 1) * 4096], AF.Square)
                    for c_ in range(NCH):
                        j = b_ * 32 + kh * 16 + c_
                        nc.tensor.matmul(
                            nacc[:, j:j + 1],
                            ysq[:, c_ * 256: c_ * 256 + 128], ones_col,
                            start=True, stop=True)
                        nc.tensor.matmul(
                            nacc[:, 64 + j:64 + j + 1],
                            ysq[:, c_ * 256 + 128: c_ * 256 + 256], ones_col,
                            start=True, stop=True)

            with tc.tile_pool(name="xin", bufs=2) as xin, \
                    tc.tile_pool(name="ysqp", bufs=2) as ysqp, \
                    tc.tile_pool(name="pcv", bufs=2, space="PSUM") as pcvp:
                for ct in (2, 0):
                    conv_ct(xin, pcvp, ct)
                norms_kh(ysqp, 0)
                for ct in (3, 1):
                    conv_ct(xin, pcvp, ct)
                norms_kh(ysqp, 1)
                for ct in (4, 5, 6, 7):
                    conv_ct(xin, pcvp, ct)

            # ---------------- ln(norms) + packed expand ----------------
            lnkq = scr.tile([128, 128], f32, tag="lnkq")
            nc.scalar.activation(lnkq, nacc, AF.Ln, bias=epsc, scale=1.0)
            # nacc layout [t, col=b*32+kh*16+c (+64 for q)] -> transpose,
            # then expand dups to packed [row=inst*16+c, t]
            Lk_pk = consts.tile([128, 128], f32, tag="Lk_pk")
            Lq_pk = consts.tile([128, 128], f32, tag="Lq_pk")
            pt = pset.tile([128, 128], f32, tag="ptr", bufs=2)
            nc.tensor.transpose(pt, lnkq, idf)
            st = scr.tile([128, 128], f32, tag="snorm")
            nc.vector.tensor_copy(st, pt)
            for qoff, dst in ((0, Lk_pk), (64, Lq_pk)):
                for b_ in range(B):
                    for kh in range(2):
                        for dup in range(2):
                            inst = b_ * 4 + kh * 2 + dup
                            nc.sync.dma_start(
                                dst[inst * 16: inst * 16 + 16, :],
                                st[qoff + b_ * 32 + kh * 16:
                                   qoff + b_ * 32 + kh * 16 + 16, :])

            # ---------------- G math part B (needs norms) ----------------
            t1 = scr.tile([128, 128], f32, tag="t1")     # G - 0.5*Lk
            nc.vector.scalar_tensor_tensor(
                t1, Lk_pk, -0.5, G, op0=OP.mult, op1=OP.add)
            Gt = consts.tile([128, 128], f32, tag="Gt")  # G + ln(beta) - 0.5*Lk
            nc.vector.tensor_sub(Gt, t1, spnb)
            Gk = consts.tile([128, 128], f32, tag="Gk")  # G + 0.5*Lk
            nc.vector.scalar_tensor_tensor(
                Gk, Lk_pk, 0.5, G, op0=OP.mult, op1=OP.add)
            t2 = scr.tile([128, 128], f32, tag="t2")
            nc.vector.scalar_tensor_tensor(
                t2, Lq_pk, -0.5, G, op0=OP.mult, op1=OP.add)
            Gq = consts.tile([128, 128], f32, tag="Gq")
            nc.vector.tensor_scalar_add(Gq, t2, -LN_HALF_DK)
            expGt = scr.tile([128, 128], f32, tag="expGt")
            nc.scalar.activation(expGt, Gt, AF.Exp)
            negexpGt = consts.tile([128, 128], f32, tag="negexpGt")
            nc.vector.tensor_scalar_mul(negexpGt, expGt, -1.0)
            decrn = consts.tile([128, 128], f32, tag="decrn")  # exp(G_C - Gk)
            nc.scalar.activation(decrn, Gk, AF.Exp, bias=G[:, 127:128],
                                 scale=-1.0)
            eGqn = consts.tile([128, 128], f32, tag="eGqn")    # exp(Gq)
            nc.scalar.activation(eGqn, Gq, AF.Exp)

            # eGC broadcast [128, 128] (col r = exp(G_C(r)) replicated)
            eGCc = scr.tile([128, 1], bf16, tag="eGCc")
            nc.scalar.activation(eGCc, G[:, 127:128], AF.Exp)
            pt1 = pset.tile([1, 128], bf16, tag="ptr", bufs=2)
            nc.tensor.transpose(pt1, eGCc, idbf)
            eGCrow = scr.tile([1, 128], bf16, tag="eGCrow")
            nc.vector.tensor_copy(eGCrow, pt1)
            pb = pset.tile([128, 128], f32, tag="ptr", bufs=2)
            nc.tensor.matmul(pb, ones1, eGCrow, start=True, stop=True)
            eGCb = consts.tile([128, 128], f32, tag="eGCb")
            nc.vector.tensor_copy(eGCb, pb)

            # transposed per-time tables: [128(t), 128(col=r)]
            beta_T = consts.tile([128, 128], f32, tag="beta_T")
            negeGt_T = consts.tile([128, 128], f32, tag="negeGt_T")
            decrn_T = consts.tile([128, 128], f32, tag="decrn_T")
            eGqn_T = consts.tile([128, 128], f32, tag="eGqn_T")
            negGk_T = consts.tile([128, 128], f32, tag="negGk_T")
            for src_t, dst, sc in ((beta_pk, beta_T, 1.0),
                                   (negexpGt, negeGt_T, 1.0),
                                   (decrn, decrn_T, 1.0),
                                   (eGqn, eGqn_T, 1.0),
                                   (Gk, negGk_T, -1.0)):
                pt2 = pset.tile([128, 128], f32, tag="ptr", bufs=2)
                nc.tensor.transpose(pt2, src_t, idf)
                if sc == 1.0:
                    nc.vector.tensor_copy(dst, pt2)
                else:
                    nc.vector.tensor_scalar_mul(dst, pt2, sc)
            # E2 bias column: -Gk(s) - kappa(r)  (kappa bounds the unmasked
            # exponent so the mask matmul can be dropped; the e^kappa is
            # restored in the U1/U2 stt scalars and the AU evacuation)
            negGkk_T = consts.tile([128, 128], f32, tag="negGkk_T")
            nc.vector.tensor_sub(negGkk_T, negGk_T, kapb)

            # hi/lo bf16 splits of Gt/Gq
            def hilo(src_t, nm):
                hi = consts.tile([128, 128], bf16, name=f"{nm}h", tag=f"{nm}h")
                nc.vector.tensor_copy(hi, src_t)
                lo = consts.tile([128, 128], bf16, name=f"{nm}l", tag=f"{nm}l")
                nc.vector.tensor_sub(lo, src_t, hi)
                return hi, lo

            Gth, Gtl = hilo(Gt, "Gt")
            Gqh, Gql = hilo(Gq, "Gq")

            # flatten hi/lo tables to [2(hi,lo), (c,i,w,t)] in SBUF so the
            # per-chunk per-pair rank-1 matmul rhs is one CONTIGUOUS
            # [2, 512] slice ([Gt_i0|Gq_i0|Gt_i1|Gq_i1]).  Own pool
            # (opened after conv scratch closed) so the stack allocator
            # reuses the freed conv region.
            flatp = ctx.enter_context(tc.tile_pool(name="flatp", bufs=1))
            GtGq_flat = flatp.tile([2, NCH * 8 * 2 * 128], bf16,
                                   tag="GtGq_flat", name="GtGq_flat")
            fl_v = GtGq_flat.rearrange("p (c i w t) -> p c i w t",
                                       c=NCH, i=8, w=2)
            for row, w, src_t in ((0, 0, Gth), (1, 0, Gtl),
                                  (0, 1, Gqh), (1, 1, Gql)):
                for i in range(8):
                    nc.sync.dma_start(
                        fl_v[row:row + 1, :, i:i + 1, w:w + 1, :],
                        src_t[i * 16:(i + 1) * 16, :])

            # ---------------- state init ----------------
            S = {}
            for p in range(4):
                s0 = consts.tile([128, 256], bf16, name=f"S{p}", tag=f"S{p}")
                nc.vector.memset(s0, 0.0)
                S[p] = s0

            pset_stack.close()

            # ---------------- chunk loop ----------------
            ps = ctx.enter_context(tc.tile_pool(name="ps", bufs=1,
                                                space="PSUM"))

            def rof(inst, c_):
                return inst * 16 + c_

            for c_ in range(NCH):
                def kqslice(kh, b_, width, off=0):
                    base = (b_ * 16 + c_) * 256 + off
                    return ykq[kh][:, base: base + width]

                # P1: decay psums [L0|A0|L1|A1] + E2 (mask-free: kappa
                # shift keeps unmasked exponents finite; mask applied as
                # 0/1 multiply in the KKQ evacuation)
                pDs = []
                for p, (b_, kh) in enumerate(pairs):
                    i0 = b_ * 4 + kh * 2
                    pD = ps.tile([128, 512], f32, tag="pD", bufs=2,
                                 name="pD")
                    nc.tensor.matmul(
                        pD, ones2,
                        GtGq_flat[:, c_ * 2048 + i0 * 256:
                                  c_ * 2048 + (i0 + 2) * 256],
                        start=True, stop=True)
                    pDs.append(pD)
                E2 = {}
                for p, (b_, kh) in enumerate(pairs):
                    for dup in range(2):
                        inst = b_ * 4 + kh * 2 + dup
                        r = rof(inst, c_)
                        e2 = work.tile([128, 256], bf16, tag="e2", bufs=8,
                                       name="e2")
                        nc.scalar.activation(
                            e2, pDs[p][:, dup * 256:(dup + 1) * 256],
                            AF.Exp, bias=negGkk_T[:, r:r + 1], scale=1.0)
                        E2[inst] = e2

                # P2: KKQ + pR psums (packed per-pair bank) + KKQ evac
                # pX layout per use:
                #   pkr:  pk [0:256], pR [256:512]
                #   pU:   pU1_d0 [0:128], pU1_d1 [128:256],
                #         pU2_d0 [256:384], pU2_d1 [384:512]
                #   pout: pQS [0:256], pAU_d0 [256:384], pAU_d1 [384:512]
                #   pS:   4 insts x [128]
                kq_sb = {}
                pkr = []
                for p, (b_, kh) in enumerate(pairs):
                    px = ps.tile([128, 512], f32, tag="pX", bufs=4,
                                 name="pkr")
                    pkr.append(px)
                    nc.tensor.matmul(px[:, 0:256], kqslice(kh, b_, 128),
                                     kqslice(kh, b_, 256),
                                     start=True, stop=True)
                    # negated + causal-masked evac so nl is a plain multiply
                    kq = work.tile([128, 256], bf16, tag="kq", bufs=4,
                                   name="kq")
                    nc.vector.scalar_tensor_tensor(
                        kq, px[:, 0:256], -1.0, mask01,
                        op0=OP.mult, op1=OP.mult)
                    kq_sb[p] = kq

                # P3: nl = E2 * (-[KK|KQ])  (bf16 2x DVE)
                nL = {}
                for p, (b_, kh) in enumerate(pairs):
                    for dup in range(2):
                        inst = b_ * 4 + kh * 2 + dup
                        nl = work.tile([128, 256], bf16, tag="nl", bufs=8,
                                       name="nl")
                        nc.vector.tensor_mul(nl, E2[inst], kq_sb[p])
                        nL[inst] = nl

                # P4: k + v transposes (packed bf16 bank) + ktl/bv (ACT)
                # pTa: pkt p0..p3 at [p*128], pVT inst0..3 at [512+hl*128]
                # pTb: pVT inst4..7 at [hl*128]
                pTa = ps.tile([128, 1024], bf16, tag="pT", bufs=2,
                              name="pTa")
                pTb = ps.tile([128, 1024], bf16, tag="pT", bufs=2,
                              name="pTb")
                ktl = {}
                for p, (b_, kh) in enumerate(pairs):
                    pkt = pTa[:, p * 128:(p + 1) * 128]
                    nc.tensor.transpose(pkt, kqslice(kh, b_, 128), idbf)
                    for dup in range(2):
                        inst = b_ * 4 + kh * 2 + dup
                        r = rof(inst, c_)
                        kt = work.tile([128, 128], bf16, tag="ktl", bufs=8,
                                       name="kt")
                        nc.scalar.mul(kt, pkt, decrn_T[:, r:r + 1])
                        ktl[inst] = kt

                bv = {}
                for b_ in range(B):
                    for hl in range(4):
                        inst = b_ * 4 + hl
                        r = rof(inst, c_)
                        if b_ == 0:
                            pVT = pTa[:, 512 + hl * 128: 640 + hl * 128]
                        else:
                            pVT = pTb[:, hl * 128:(hl + 1) * 128]
                        nc.tensor.transpose(
                            pVT, ytv[hl][:, b_ * T + c_ * C:
                                         b_ * T + (c_ + 1) * C], idbf)
                        b1 = work.tile([128, 128], bf16, tag="bv", bufs=8,
                                       name="b1")
                        nc.scalar.mul(b1, pVT, beta_T[:, r:r + 1])
                        bv[inst] = b1

                # P6: pR (pair, into pkr bank) + R (DVE, pair tile halves)
                for p, (b_, kh) in enumerate(pairs):
                    nc.tensor.matmul(pkr[p][:, 256:512],
                                     kqslice(kh, b_, 128), S[p],
                                     start=True, stop=True)
                R = {}
                Rp = {}
                for p, (b_, kh) in enumerate(pairs):
                    Rpair = work.tile([128, 256], bf16, tag="R", bufs=8,
                                      name="Rpair")
                    Rp[p] = Rpair
                    for dup in range(2):
                        inst = b_ * 4 + kh * 2 + dup
                        r = rof(inst, c_)
                        nc.vector.scalar_tensor_tensor(
                            Rpair[:, dup * 128:(dup + 1) * 128],
                            pkr[p][:, 256 + dup * 128:
                                   256 + (dup + 1) * 128],
                            negeGt_T[:, r:r + 1], bv[inst],
                            op0=OP.mult, op1=OP.add)
                        R[inst] = Rpair[:, dup * 128:(dup + 1) * 128]

                # P7/P8: Horner — U1 = nL@R + R, U2 = nL@U1 + R
                # (pair-wide DVE stt evacuations)
                pUs = []
                U1 = {}
                for p, (b_, kh) in enumerate(pairs):
                    pU = ps.tile([128, 512], f32, tag="pX", bufs=4,
                                 name="pU")
                    pUs.append(pU)
                    for dup in range(2):
                        inst = b_ * 4 + kh * 2 + dup
                        nc.tensor.matmul(pU[:, dup * 128:(dup + 1) * 128],
                                         nL[inst][:, 0:128], R[inst],
                                         start=True, stop=True)
                    r0 = rof(b_ * 4 + kh * 2, c_)
                    u1p = work.tile([128, 256], bf16, tag="U1", bufs=8,
                                    name="u1p")
                    nc.vector.scalar_tensor_tensor(
                        u1p, pU[:, 0:256], ekapb[:, r0:r0 + 1], Rp[p],
                        op0=OP.mult, op1=OP.add)
                    for dup in range(2):
                        inst = b_ * 4 + kh * 2 + dup
                        U1[inst] = u1p[:, dup * 128:(dup + 1) * 128]
                U2 = {}
                for p, (b_, kh) in enumerate(pairs):
                    for dup in range(2):
                        inst = b_ * 4 + kh * 2 + dup
                        nc.tensor.matmul(
                            pUs[p][:, 256 + dup * 128: 256 + (dup + 1) * 128],
                            nL[inst][:, 0:128], U1[inst],
                            start=True, stop=True)
                    r0 = rof(b_ * 4 + kh * 2, c_)
                    u2p = work.tile([128, 256], bf16, tag="U2", bufs=8,
                                    name="u2p")
                    nc.vector.scalar_tensor_tensor(
                        u2p, pUs[p][:, 256:512], ekapb[:, r0:r0 + 1], Rp[p],
                        op0=OP.mult, op1=OP.add)
                    for dup in range(2):
                        inst = b_ * 4 + kh * 2 + dup
                        U2[inst] = u2p[:, dup * 128:(dup + 1) * 128]

                # P9: output O = eGqn*(q S) - (-A)U  (pout bank per pair)
                pouts = []
                for p, (b_, kh) in enumerate(pairs):
                    po = ps.tile([128, 512], f32, tag="pX", bufs=4,
                                 name="pout")
                    pouts.append(po)
                    nc.tensor.matmul(po[:, 0:256],
                                     kqslice(kh, b_, 128, off=128),
                                     S[p], start=True, stop=True)
                    for dup in range(2):
                        inst = b_ * 4 + kh * 2 + dup
                        nc.tensor.matmul(
                            po[:, 256 + dup * 128: 256 + (dup + 1) * 128],
                            nL[inst][:, 128:256], U2[inst],
                            start=True, stop=True)
                for p, (b_, kh) in enumerate(pairs):
                    r0 = rof(b_ * 4 + kh * 2, c_)
                    aup = work.tile([128, 256], bf16, tag="AU", bufs=4,
                                    name="aup")
                    nc.scalar.mul(aup, pouts[p][:, 256:512],
                                  ekapb[:, r0:r0 + 1])
                    for dup in range(2):
                        hl = kh * 2 + dup
                        inst = b_ * 4 + hl
                        r = rof(inst, c_)
                        O_sb = work.tile([128, 128], bf16, tag="O", bufs=4,
                                         name="O_sb")
                        nc.vector.scalar_tensor_tensor(
                            O_sb, pouts[p][:, dup * 128:(dup + 1) * 128],
                            eGqn_T[:, r:r + 1],
                            aup[:, dup * 128:(dup + 1) * 128],
                            op0=OP.mult, op1=OP.subtract)
                        nc.sync.dma_start(
                            out_d[b_, c_ * C:(c_ + 1) * C,
                                  hl * DV:(hl + 1) * DV], O_sb)

                # P10: state update (pS packed 4-per-bank, 2 banks/chunk)
                Snew = {}
                for p, (b_, kh) in enumerate(pairs):
                    Snew[p] = spool.tile([128, 256], bf16, tag=f"Sn{p}",
                                         name="Snew")
                for half in range(2):
                    pSb = ps.tile([128, 512], f32, tag="pX", bufs=4,
                                  name="pSb")
                    for pi in range(2):
                        p = half * 2 + pi
                        b_, kh = pairs[p]
                        for dup in range(2):
                            inst = b_ * 4 + kh * 2 + dup
                            r = rof(inst, c_)
                            sl = pSb[:, (pi * 2 + dup) * 128:
                                     (pi * 2 + dup + 1) * 128]
                            nc.tensor.matmul(sl, ktl[inst], U2[inst],
                                             start=True, stop=True)
                            nc.vector.scalar_tensor_tensor(
                                Snew[p][:, dup * 128:(dup + 1) * 128],
                                S[p][:, dup * 128:(dup + 1) * 128],
                                eGCb[:, r:r + 1], sl,
                                op0=OP.mult, op1=OP.add)
                for p in range(4):
                    S[p] = Snew[p]
    nc.compile()
    return nc


# ---------------------------------------------------------------------------
# host-side prep
# ---------------------------------------------------------------------------

def host_inputs(core, mixed_qkv, a, b, conv_weight, conv_bias, A_log, dt_bias):
    import ml_dtypes
    bf = ml_dtypes.bfloat16
    f32 = np.float32

    chsel = np.r_[256 * core: 256 * core + 256,
                  2048 + 256 * core: 2048 + 256 * core + 256,
                  4096 + 512 * core: 4096 + 512 * core + 512]
    xs = mixed_qkv[:, :, chsel]                     # [B,T,1024]
    ws = conv_weight[chsel].astype(f32)             # [1024,4]
    cbs = conv_bias[chsel].astype(f32)

    x_cm = np.zeros((8, 128, B * TPAD), dtype=bf)
    xt = np.ascontiguousarray(xs.transpose(2, 0, 1))  # [1024,B,T]
    for b_ in range(B):
        x_cm[:, :, b_ * TPAD + 3: (b_ + 1) * TPAD] = \
            xt[:, b_, :].reshape(8, 128, T).astype(bf)

    wdiag = np.zeros((8, KW, 128, 128), dtype=bf)
    wr = ws.reshape(8, 128, KW)
    di = np.arange(128)
    for t_ in range(8):
        for j in range(KW):
            wdiag[t_, j, di, di] = wr[t_, :, j].astype(bf)

    cb_t = np.ascontiguousarray(cbs.reshape(8, 128).T).astype(f32)  # [128,8]

    a_pk = np.zeros((128, 128), f32)
    b_pk = np.zeros((128, 128), f32)
    dtb = np.zeros((128, 1), f32)
    negea = np.zeros((128, 1), f32)
    for b_ in range(B):
        for hl in range(4):
            hg = 4 * core + hl
            inst = b_ * 4 + hl
            a_pk[inst * 16: (inst + 1) * 16] = a[b_, :, hg].reshape(NCH, C)
            b_pk[inst * 16: (inst + 1) * 16] = b[b_, :, hg].reshape(NCH, C)
            dtb[inst * 16: (inst + 1) * 16] = dt_bias[hg]
            negea[inst * 16: (inst + 1) * 16] = -np.exp(A_log[hg])

    idx = np.arange(C)
    maskS = np.where(idx[:, None] < idx[None, :], 0.0, -1e6).astype(bf)
    maskI = np.where(idx[:, None] <= idx[None, :], 0.0, -1e6).astype(bf)
    maskS01 = (idx[:, None] < idx[None, :]).astype(bf)
    maskI01 = (idx[:, None] <= idx[None, :]).astype(bf)
    idbf = np.eye(128, dtype=bf)
    idf32 = np.eye(128, dtype=f32)

    wcol_t = np.ascontiguousarray(
        wr.transpose(1, 0, 2).reshape(128, 32)).astype(f32)  # [p, ct*4+j]

    # exponent shift kappa per (pair, chunk): bounds the unmasked decay
    # exponent max_t G - min_s G (+8 margin for the log-norm folds) to 80
    # so exp stays finite in f32/bf16 without the -1e6 mask matmul
    kapv = np.zeros((8, NCH), f32)
    for b_ in range(B):
        for hl in range(4):
            hg = 4 * core + hl
            g = (-np.exp(np.float64(A_log[hg])) *
                 np.logaddexp(0.0, a[b_, :, hg].astype(np.float64)
                              + dt_bias[hg]))
            Gc = np.cumsum(g.reshape(NCH, C), axis=1)
            mx = Gc.max(1) - Gc.min(1) + 8.0
            kapv[b_ * 4 + hl] = np.maximum(0.0, mx - 80.0)
    for b_ in range(B):
        for kh in range(2):
            i0 = b_ * 4 + kh * 2
            m = np.maximum(kapv[i0], kapv[i0 + 1])
            kapv[i0] = kapv[i0 + 1] = m
    kapcol = np.zeros(128, f32)
    for inst in range(8):
        kapcol[inst * 16:(inst + 1) * 16] = kapv[inst]
    kapb = np.broadcast_to(kapcol[None, :], (128, 128)).astype(f32)
    ekapb = np.exp(kapb).astype(f32)

    pf32 = np.concatenate(
        [a_pk, b_pk, dtb, negea, cb_t, idf32, wcol_t, kapb, ekapb],
        axis=1).astype(f32)
    # maskSI2 ordered [S|I|S|I] to match the pD layout [L0|A0|L1|A1]
    pbf = np.ascontiguousarray(np.concatenate(
        [maskS, maskI, idbf, maskS, maskI, maskS, maskI,
         maskS01, maskI01], axis=1)).astype(bf)
    return {"x_cm": x_cm, "wdiag": wdiag, "pf32": pf32, "pbf": pbf}


_CACHED = {}


def _get_program():
    if "nc" not in _CACHED:
        _CACHED["nc"] = build_program()
    return _CACHED["nc"]


def kernel(mixed_qkv, a, b, conv_weight, conv_bias, A_log, dt_bias, trace=False):
    f32 = np.float32
    mixed_qkv = np.asarray(mixed_qkv, f32)
    a = np.asarray(a, f32)
    b = np.asarray(b, f32)
    conv_weight = np.asarray(conv_weight, f32)
    conv_bias = np.asarray(conv_bias, f32)
    A_log = np.asarray(A_log, f32)
    dt_bias = np.asarray(dt_bias, f32)

    from concourse import bass_utils

    nc = _get_program()
    in_maps = [host_inputs(core, mixed_qkv, a, b, conv_weight, conv_bias,
                           A_log, dt_bias) for core in range(NCORE)]
    res = bass_utils.run_bass_kernel_spmd(
        nc, in_maps, core_ids=list(range(NCORE)), trace=trace)

    out = np.empty((B, T, HV * DV), f32)
    for core in range(NCORE):
        out[:, :, 512 * core: 512 * (core + 1)] = \
            res.results[core]["out"].astype(f32)
    if trace:
        return out, res
    return out
